# revision 37
# baseline (speedup 1.0000x reference)
"""Trainium2 Bass kernel for the Actor sampling module (nn_Actor_47588237640247).

Strategy: pure data-parallel across 8 NeuronCores (8 samples each). Per core:
  Phase 1: choice features cf = relu(l1(memory)) computed once into SBUF in a
           transposed layout cfT[h, (t, b, k)] (fp32, ~12.6 MB).
  Scan:    511 fully-unrolled steps, run as TWO interleaved independent chains
           (samples 0-3 / 4-7) so the engines overlap the chains' serial
           latencies. Everything lives transposed (feature dim on partitions,
           (b,k) on the free axis); per step and half:
             hid = relu(P1cf @ cf_t + P1h @ h + b)          (PE x2 + ACT relu)
             raw = p2 @ hid                                  (PE -> [1,24])
             noisy = max(raw, -p2b) + (gumbel+maskneg+p2b)   (DVE stt, staged out)
             onehot = (noisy >= rowmax)                      (DVE reduce + cmp)
             chosen = sum_k onehot * cf_t                    (PE bcast + DVE x2)
             gates  = bias + Wcomb @ h + Wcf @ chosen        (PE, gates transposed)
             LSTM cell                                       (ACT sigmoid/tanh + DVE)
           Per-step rows (noisy scores, onehot) are staged in [1, 8*48] blocks
           and DMA'd every 8 steps into rotated [128, 4*48] buffers.
  End:     logits = noisy - gumbel; batched log-softmax gather (exp/ln on ACT)
           + masked sum -> log_probs; argmax decoded from onehot -> idx.
Host side only reshapes/transposes inputs and bakes gumbel+mask+bias tensors.
"""

import os
import sys

import numpy as np

for _p in ("/opt/trn_rl_repo", "/root/.axon_site/_ro/trn_rl_repo"):
    if os.path.isdir(_p) and _p not in sys.path:
        sys.path.append(_p)

B, L, E, H, C = 64, 512, 256, 128, 128
T = L - 1  # 511
NCORES = 8
BL = B // NCORES  # 8 samples per core
K6 = 6
BK = BL * K6  # 48
HB = BL // 2  # 4 samples per half-chain
HK = HB * K6  # 24
NEG = np.float32(-1.0e30)


def _build_program(p2b_val: float, T_: int):
    import concourse.bass as bass
    import concourse.tile as tile
    from concourse import bacc, mybir

    f32 = mybir.dt.float32
    i32 = mybir.dt.int32
    AL = mybir.AluOpType
    AF = mybir.ActivationFunctionType
    AX = mybir.AxisListType

    NT_ = (T_ + 127) // 128  # rotated-layout column blocks

    nc = bacc.Bacc(
        "TRN2", target_bir_lowering=False, debug=False, num_devices=NCORES
    )

    d_memT = nc.dram_tensor("memT", [E, BL * T_], f32, kind="ExternalInput")
    d_gp2 = nc.dram_tensor("gp2", [1, T_ * BK], f32, kind="ExternalInput")
    d_gmr = nc.dram_tensor("gmr", [128, NT_ * BK], f32, kind="ExternalInput")
    d_mb = nc.dram_tensor("mb", [128, NT_ * BL], f32, kind="ExternalInput")
    d_kp = nc.dram_tensor("kp", [128, NT_ * BK], f32, kind="ExternalInput")
    d_l1wT = nc.dram_tensor("l1wT", [E, 6 * H], f32, kind="ExternalInput")
    d_l1b6 = nc.dram_tensor("l1b6", [128, 6], f32, kind="ExternalInput")
    d_p1cfT = nc.dram_tensor("p1cfT", [128, 128], f32, kind="ExternalInput")
    d_p1hT = nc.dram_tensor("p1hT", [128, 128], f32, kind="ExternalInput")
    d_p1b = nc.dram_tensor("p1b", [128, 1], f32, kind="ExternalInput")
    d_p2wT = nc.dram_tensor("p2wT", [128, 1], f32, kind="ExternalInput")
    d_wcfT = nc.dram_tensor("wcfT", [128, 4 * C], f32, kind="ExternalInput")
    d_wcoT = nc.dram_tensor("wcoT", [128, 4 * C], f32, kind="ExternalInput")
    d_bcr = nc.dram_tensor("bcr", [4, C], f32, kind="ExternalInput")
    d_bcT = nc.dram_tensor("bcT", [C, 4], f32, kind="ExternalInput")
    d_e46 = nc.dram_tensor("e46", [4, 4 * BK], f32, kind="ExternalInput")
    d_id128 = nc.dram_tensor("id128", [128, 128], f32, kind="ExternalInput")
    d_oidx = nc.dram_tensor("out_idx", [BL, T_], i32, kind="ExternalOutput")
    d_olp = nc.dram_tensor("out_lp", [BL, 1], f32, kind="ExternalOutput")

    with tile.TileContext(nc) as tc:
        with (
            tc.tile_pool(name="persist", bufs=1) as P,
            tc.tile_pool(name="blk", bufs=3) as BP,
            tc.tile_pool(name="work", bufs=4) as W,
        ):
            # ---- persistent SBUF tiles
            l1b6 = P.tile([128, 6], f32, tag="l1b6")
            cft = P.tile([128, T_, BK], f32, tag="cft")
            p1cf = P.tile([128, 128], f32, tag="p1cf")
            p1h = P.tile([128, 128], f32, tag="p1h")
            p1b = P.tile([128, 1], f32, tag="p1b")
            p2w = P.tile([128, 1], f32, tag="p2w")
            wcf = P.tile([128, 4 * C], f32, tag="wcf")
            wco = P.tile([128, 4 * C], f32, tag="wco")
            bcr4 = P.tile([4, C], f32, tag="bcr4")
            bcT = P.tile([C, 4], f32, tag="bcT")
            e46 = P.tile([4, 4 * BK], f32, tag="e46")
            id128 = P.tile([128, 128], f32, tag="id128")
            gmr = P.tile([128, NT_ * BK], f32, tag="gmr")
            mbs = P.tile([128, NT_ * BL], f32, tag="mbs")
            kps = P.tile([128, NT_ * BK], f32, tag="kps")
            scb = P.tile([128, NT_, BK], f32, tag="scb")
            oneb = P.tile([128, NT_, BK], f32, tag="oneb")

            ones1128 = P.tile([1, 128], f32, tag="ones1128")
            ones128 = P.tile([128, 1], f32, tag="ones128")
            zero128 = P.tile([128, 1], f32, tag="zero128")

            # ---- input DMAs
            nc.sync.dma_start(out=l1b6[:], in_=d_l1b6.ap())
            nc.sync.dma_start(out=p1cf[:], in_=d_p1cfT.ap())
            nc.sync.dma_start(out=p1h[:], in_=d_p1hT.ap())
            nc.sync.dma_start(out=p1b[:], in_=d_p1b.ap())
            nc.sync.dma_start(out=p2w[:], in_=d_p2wT.ap())
            nc.sync.dma_start(out=wcf[:], in_=d_wcfT.ap())
            nc.sync.dma_start(out=wco[:], in_=d_wcoT.ap())
            nc.sync.dma_start(out=bcr4[:], in_=d_bcr.ap())
            nc.sync.dma_start(out=bcT[:], in_=d_bcT.ap())
            nc.sync.dma_start(out=e46[:], in_=d_e46.ap())
            nc.sync.dma_start(out=id128[:], in_=d_id128.ap())
            nc.sync.dma_start(out=gmr[:], in_=d_gmr.ap())
            nc.sync.dma_start(out=mbs[:], in_=d_mb.ap())
            nc.sync.dma_start(out=kps[:], in_=d_kp.ap())


            nc.vector.memset(ones1128[:], 1.0)
            nc.vector.memset(ones128[:], 1.0)
            nc.vector.memset(zero128[:], 0.0)
            nc.vector.memset(scb[:], 0.0)
            nc.vector.memset(oneb[:], 0.0)

            # ---- phase 1: cf = relu(l1 @ mem + b), scattered into cfT layout
            with (
                tc.tile_pool(name="ph1sbuf", bufs=1) as P1S,
                tc.tile_pool(name="ph1psum", bufs=4, space="PSUM") as PP1,
            ):
                memT0 = P1S.tile([128, BL * T_], f32, tag="memT0")
                memT1 = P1S.tile([128, BL * T_], f32, tag="memT1")
                l1w0 = P1S.tile([128, 6 * H], f32, tag="l1w0")
                l1w1 = P1S.tile([128, 6 * H], f32, tag="l1w1")
                nc.sync.dma_start(out=memT0[:], in_=d_memT.ap()[0:128, :])
                nc.sync.dma_start(out=memT1[:], in_=d_memT.ap()[128:256, :])
                nc.sync.dma_start(out=l1w0[:], in_=d_l1wT.ap()[0:128, :])
                nc.sync.dma_start(out=l1w1[:], in_=d_l1wT.ap()[128:256, :])
                for b in range(BL):
                    for k in range(K6):
                        pC = PP1.tile([128, T_], f32, tag="pC")
                        nc.tensor.matmul(
                            pC[:],
                            l1w0[:, k * H : (k + 1) * H],
                            memT0[:, b * T_ : (b + 1) * T_],
                            start=True,
                            stop=False,
                        )
                        nc.tensor.matmul(
                            pC[:],
                            l1w1[:, k * H : (k + 1) * H],
                            memT1[:, b * T_ : (b + 1) * T_],
                            start=False,
                            stop=True,
                        )
                        nc.vector.tensor_scalar(
                            cft[:, :, b * K6 + k],
                            pC[:],
                            l1b6[:, k : k + 1],
                            0.0,
                            AL.add,
                            AL.max,
                        )

            # ---- scan: full-width chain; speculative LSTM over all 6 candidates
            # off-chain: gates_k = bias + Wco@h + (Wcf@cf_k pre-batched); LSTM for
            # all k; chain only: hid-relu -> scores -> noisy -> argmax -> select.
            MB = 16  # steps per staging block
            with tc.tile_pool(name="spsum", bufs=2, space="PSUM") as SP:
                hsel = W.tile([128, 2 * BL], f32, tag="hcsel", name="hsel0")
                nc.vector.memset(hsel[:], 0.0)
                csel0 = W.tile([128, BL], f32, tag="csel0", name="csel0")
                nc.vector.memset(csel0[:], 0.0)
                csel = csel0
                nb8 = None
                one16 = None
                gpblk = None
                gcf16 = None
                for t in range(T_):
                    m16 = t % MB
                    if m16 == 0:
                        nb = min(MB, T_ - t)
                        gpblk = BP.tile([1, MB * BK], f32, tag="gpblk")
                        nc.sync.dma_start(
                            out=gpblk[:, : nb * BK],
                            in_=d_gp2.ap()[:, t * BK : (t + nb) * BK],
                        )
                        nb8 = BP.tile([1, MB * BK], f32, tag="nb8")
                        one16 = BP.tile([1, MB * BK], f32, tag="one16")
                        # pre-batch Wcf @ cf for the block: gcf16[t', (g,b,k)]
                        gcf16 = BP.tile([128, MB, 4 * BK], f32, tag="gcf16", bufs=2)
                        for g in range(4):
                            for c2 in range(2):
                                t0c = c2 * 8
                                ch = min(8, nb - t0c)
                                if ch <= 0:
                                    continue
                                psC = SP.tile(
                                    [128, 8 * BK], f32, tag="psC", bufs=1, name="psC"
                                )
                                nc.tensor.matmul(
                                    psC[:, : ch * BK],
                                    wcf[:, g * C : (g + 1) * C],
                                    cft[:, t + t0c : t + t0c + ch, :].rearrange(
                                        "p a k -> p (a k)"
                                    ),
                                    start=True,
                                    stop=True,
                                )
                                nc.scalar.activation(
                                    gcf16[
                                        :, t0c : t0c + ch, g * BK : (g + 1) * BK
                                    ],
                                    psC[:, : ch * BK].rearrange(
                                        "p (a k) -> p a k", k=BK
                                    ),
                                    AF.Identity,
                                    bias=zero128[:],
                                    scale=1.0,
                                )

                    cft_t = cft[:, t, :]  # [128, 48]
                    hT48b = (
                        hsel[:, 0:BL].unsqueeze(2).to_broadcast([128, BL, K6])
                    )
                    hT8 = hsel[:, 0:BL]

                    # hid pre-activation (chain) + gate psums (off-chain)
                    pA = SP.tile([128, BK], f32, tag="pA", bufs=2)
                    nc.tensor.matmul(pA[:], p1cf[:], cft_t, start=True, stop=False)
                    nc.tensor.matmul(
                        pA.rearrange("p (b k) -> p b k", k=K6),
                        p1h[:],
                        hT48b,
                        start=False,
                        stop=True,
                    )

                    pG48 = SP.tile([128, 4 * BK], f32, tag="pG48", bufs=2)
                    nc.tensor.matmul(pG48[:], bcr4[:], e46[:], start=True, stop=False)
                    hT8b = hT8.unsqueeze(2).to_broadcast([128, BL, K6])
                    for g in range(4):
                        nc.tensor.matmul(
                            pG48[:, g * BK : (g + 1) * BK].rearrange(
                                "p (b k) -> p b k", k=K6
                            ),
                            wco[:, g * C : (g + 1) * C],
                            hT8b,
                            start=False,
                            stop=False,
                        )
                    nc.tensor.matmul(
                        pG48[:], id128[:], gcf16[:, m16, :], start=False, stop=True
                    )

                    # speculative LSTM for all 6 candidates (off-chain)
                    # i/f/o gate weights pre-scaled 0.5 on host: sigmoid(x) =
                    # 0.5*tanh(x/2)+0.5 -> ONE Tanh covers all four gates.
                    th = W.tile([128, 4 * BK], f32, tag="th")
                    nc.scalar.activation(
                        th[:], pG48[:], AF.Tanh, bias=zero128[:], scale=1.0
                    )
                    sig = W.tile([128, 3 * BK], f32, tag="sig")
                    nc.vector.tensor_scalar(
                        sig[:], th[:, 0 : 3 * BK], 0.5, 0.5, AL.mult, AL.add
                    )
                    hc_cand = W.tile([128, 2 * BK], f32, tag="hc_cand")
                    t1 = W.tile([128, BK], f32, tag="t1")
                    nc.vector.tensor_mul(
                        t1.rearrange("p (b k) -> p b k", k=K6),
                        sig[:, BK : 2 * BK].rearrange("p (b k) -> p b k", k=K6),
                        csel.unsqueeze(2).to_broadcast([128, BL, K6]),
                    )
                    t2 = W.tile([128, BK], f32, tag="t2")
                    nc.vector.tensor_mul(t2[:], sig[:, 0:BK], th[:, 3 * BK : 4 * BK])
                    nc.vector.tensor_add(hc_cand[:, BK : 2 * BK], t1[:], t2[:])
                    tcs = W.tile([128, BK], f32, tag="tcs")
                    nc.scalar.activation(
                        tcs[:], hc_cand[:, BK : 2 * BK], AF.Tanh, bias=zero128[:], scale=1.0
                    )
                    nc.vector.tensor_mul(
                        hc_cand[:, 0:BK], sig[:, 2 * BK : 3 * BK], tcs[:]
                    )

                    # chain: hid -> scores -> noisy -> argmax -> select h,c
                    hid = W.tile([128, BK], f32, tag="hid")
                    nc.vector.tensor_scalar(
                        hid[:], pA[:], p1b[:], 0.0, AL.add, AL.max
                    )
                    pS = SP.tile([1, BK], f32, tag="pS", bufs=2)
                    nc.tensor.matmul(pS[:], p2w[:], hid[:], start=True, stop=True)
                    ms = slice(m16 * BK, (m16 + 1) * BK)
                    nc.vector.scalar_tensor_tensor(
                        nb8[:, ms], pS[:], -p2b_val, gpblk[:, ms], AL.max, AL.add
                    )
                    nv = nb8[:, ms].rearrange("p (b k) -> p b k", k=K6)
                    nmax = W.tile([1, BL], f32, tag="nmax")
                    nc.vector.reduce_max(nmax[:], nv, axis=AX.X)
                    one_v = one16[:, ms].rearrange("p (b k) -> p b k", k=K6)
                    nmax_b = nmax.unsqueeze(2).to_broadcast([1, BL, K6])
                    nc.vector.tensor_tensor(one_v, nv, nmax_b, op=AL.is_ge)
                    pO2 = SP.tile([128, 2 * BK], f32, tag="pO2", bufs=1)
                    nc.tensor.matmul(
                        pO2[:],
                        ones1128[:],
                        one16[:, ms]
                        .rearrange("p (a k) -> p a k", a=1)
                        .to_broadcast([1, 2, BK]),
                        start=True,
                        stop=True,
                    )
                    tmp96 = W.tile([128, 2 * BK], f32, tag="tmp96")
                    nc.vector.tensor_mul(tmp96[:], hc_cand[:], pO2[:])
                    hcsel = W.tile([128, 2 * BL], f32, tag="hcsel")
                    nc.vector.reduce_sum(
                        hcsel[:],
                        tmp96.rearrange("p (a b k) -> p a b k", a=2, k=K6),
                        axis=AX.X,
                    )
                    hsel = hcsel
                    csel = hcsel[:, BL : 2 * BL]

                    if m16 == MB - 1 or t == T_ - 1:
                        nb = m16 + 1
                        p0 = (t - m16) % 128
                        jj = (t - m16) // 128
                        nc.sync.dma_start(
                            out=scb[p0 : p0 + nb, jj, :],
                            in_=nb8[:, : nb * BK].rearrange(
                                "p (n k) -> p n k", k=BK
                            ),
                        )
                        nc.sync.dma_start(
                            out=oneb[p0 : p0 + nb, jj, :],
                            in_=one16[:, : nb * BK].rearrange(
                                "p (n k) -> p n k", k=BK
                            ),
                        )

            # ---- end phase: log-probs and indices
            with (
                tc.tile_pool(name="endp", bufs=1) as EP,
                tc.tile_pool(name="endpsum", bufs=1, space="PSUM") as PEP,
            ):
                scb_f = scb.rearrange("p j k -> p (j k)")
                oneb_f = oneb.rearrange("p j k -> p (j k)")
                logits = EP.tile([128, NT_ * BK], f32, tag="logits")
                nc.vector.tensor_sub(logits[:], scb_f, gmr[:])
                expv = EP.tile([128, NT_ * BK], f32, tag="expv")
                nc.scalar.activation(
                    expv[:], logits[:], AF.Exp, bias=zero128[:], scale=1.0
                )
                esum = EP.tile([128, NT_ * BL], f32, tag="esum")
                nc.vector.reduce_sum(
                    esum[:],
                    expv.rearrange("p (j b k) -> p j b k", b=BL, k=K6),
                    axis=AX.X,
                )
                lse = EP.tile([128, NT_ * BL], f32, tag="lse")
                nc.scalar.activation(
                    lse[:], esum[:], AF.Ln, bias=zero128[:], scale=1.0
                )
                selp = EP.tile([128, NT_ * BK], f32, tag="selp")
                nc.vector.tensor_mul(selp[:], logits[:], oneb_f)
                sel = EP.tile([128, NT_ * BL], f32, tag="sel")
                nc.vector.reduce_sum(
                    sel[:],
                    selp.rearrange("p (j b k) -> p j b k", b=BL, k=K6),
                    axis=AX.X,
                )
                diff = EP.tile([128, NT_ * BL], f32, tag="diff")
                nc.vector.tensor_sub(diff[:], sel[:], lse[:])
                masked = EP.tile([128, NT_ * BL], f32, tag="masked")
                nc.vector.tensor_mul(masked[:], diff[:], mbs[:])
                part = EP.tile([128, BL], f32, tag="part")
                nc.vector.reduce_sum(
                    part[:],
                    masked.rearrange("p (j b) -> p b j", b=BL),
                    axis=AX.X,
                )
                accp = PEP.tile([BL, 1], f32, tag="accp")
                nc.tensor.matmul(accp[:], part[:], ones128[:], start=True, stop=True)
                lp_sb = EP.tile([BL, 1], f32, tag="lp_sb")
                nc.vector.tensor_copy(lp_sb[:], accp[:])
                nc.sync.dma_start(out=d_olp.ap(), in_=lp_sb[:])

                ksel = EP.tile([128, NT_ * BK], f32, tag="ksel")
                nc.vector.tensor_mul(ksel[:], oneb_f, kps[:])
                idxf = EP.tile([128, NT_ * BL], f32, tag="idxf")
                nc.vector.reduce_sum(
                    idxf[:],
                    ksel.rearrange("p (j b k) -> p j b k", b=BL, k=K6),
                    axis=AX.X,
                )
                idxi = EP.tile([128, NT_ * BL], i32, tag="idxi")
                nc.vector.tensor_copy(idxi[:], idxf[:])
                for j in range(NT_):
                    pmax = min(128, T_ - j * 128)
                    nc.sync.dma_start(
                        out=d_oidx.ap()[:, j * 128 : j * 128 + pmax].transpose(
                            [1, 0]
                        ),
                        in_=idxi[0:pmax, j * BL : (j + 1) * BL],
                    )

    nc.compile()
    return nc


def _prep_shared(inputs, T_):
    """Host-side weight prep (shared across cores)."""
    f = np.float32
    l1_w = inputs["l1_w"].astype(f)
    p1_w = inputs["p1_w"].astype(f)
    w_ih = inputs["w_ih"].astype(f)
    w_hh = inputs["w_hh"].astype(f)
    bc = (inputs["b_ih"] + inputs["b_hh"]).astype(f)
    NT_ = (T_ + 127) // 128
    gorder = [0, 1, 3, 2]  # i, f, o, g
    wcfT = np.empty((128, 4 * C), f)
    wcoT = np.empty((128, 4 * C), f)
    bcr = np.empty((4, C), f)
    for gi, g in enumerate(gorder):
        rows = slice(g * C, (g + 1) * C)
        s5 = f(0.5) if gi < 3 else f(1.0)  # sigmoid-as-tanh pre-scale (i,f,o)
        wcfT[:, gi * C : (gi + 1) * C] = s5 * w_ih[rows, :H].T
        wcoT[:, gi * C : (gi + 1) * C] = s5 * (w_ih[rows, H:] + w_hh[rows, :]).T
        bcr[gi, :] = s5 * bc[rows]
    kcol = np.tile(np.arange(K6, dtype=f), BL)  # [48]
    kp = np.broadcast_to(kcol, (128, NT_, BK)).reshape(128, NT_ * BK).copy()
    bcT = bcr.T.copy()
    return {
        "bcT": bcT,
        "l1wT": np.ascontiguousarray(l1_w.T),
        "l1b6": np.ascontiguousarray(inputs["l1_b"].astype(f).reshape(6, 128).T),
        "p1cfT": np.ascontiguousarray(p1_w[:, :H].T),
        "p1hT": np.ascontiguousarray(p1_w[:, H:].T),
        "p1b": inputs["p1_b"].astype(f).reshape(128, 1).copy(),
        "p2wT": np.ascontiguousarray(inputs["p2_w"].astype(f).T),
        "wcfT": wcfT,
        "wcoT": wcoT,
        "bcr": bcr,
        "e46": np.kron(np.eye(4, dtype=f), np.ones((1, BK), f)),
        "id128": np.eye(128, dtype=f),
        "kp": kp,
    }


def _rot(x_t, T_, width):
    """[T, width] -> rotated [128, NT*width] (partition = t%128, block = t//128)."""
    f = np.float32
    NT_ = (T_ + 127) // 128
    Xp = np.zeros((NT_ * 128, width), f)
    Xp[:T_] = x_t
    return np.ascontiguousarray(
        Xp.reshape(NT_, 128, width).transpose(1, 0, 2).reshape(128, NT_ * width)
    )


def _prep_core(inputs, ci, T_):
    f = np.float32
    p2b = f(np.asarray(inputs["p2_b"]).reshape(-1)[0])
    sl = slice(ci * BL, (ci + 1) * BL)
    mem = inputs["memory"][sl, 2 : 2 + T_, :].astype(f)  # [8, T, 256]
    memT = np.ascontiguousarray(mem.transpose(2, 0, 1).reshape(E, BL * T_))
    mask_t = inputs["mask"][sl, 1 : 1 + T_, :]  # [8, T, 6]
    SMt = np.where(mask_t, f(0.0), NEG).astype(f).transpose(1, 0, 2)  # [T, 8, 6]
    gum = inputs["gumbel"][:T_, sl, :].astype(f)  # [T, 8, 6]
    gp2 = (gum + SMt + p2b).reshape(1, T_ * BK).astype(f)
    gmr = _rot(gum.reshape(T_, BK), T_, BK)
    length = inputs["length"][sl].astype(np.int64)
    Mt = (length[None, :] > (np.arange(T_) + 1)[:, None]).astype(f)  # [T, 8]
    mb = _rot(Mt, T_, BL)
    return {"memT": memT, "gp2": gp2, "gmr": gmr, "mb": mb}


def _make_in_maps(inputs, T_):
    shared = _prep_shared(inputs, T_)
    return [dict(shared, **_prep_core(inputs, ci, T_)) for ci in range(NCORES)]


LAST_RESULTS = None


def kernel(**inputs):
    global LAST_RESULTS
    from concourse.bass_utils import run_bass_kernel_spmd

    inputs = {k: np.asarray(v) for k, v in inputs.items()}
    in_maps = _make_in_maps(inputs, T)
    nc = _build_program(float(np.asarray(inputs["p2_b"]).reshape(-1)[0]), T)
    res = run_bass_kernel_spmd(nc, in_maps, core_ids=list(range(NCORES)))
    LAST_RESULTS = res
    idx = np.concatenate([r["out_idx"] for r in res.results], axis=0)
    lp = np.concatenate([r["out_lp"][:, 0] for r in res.results], axis=0)
    return idx.astype(np.int32), lp.astype(np.float32)


# revision 44
# speedup vs baseline: 1.0704x; 1.0704x over previous
"""Trainium2 Bass kernel for the Actor sampling module (nn_Actor_47588237640247).

Strategy: pure data-parallel across 8 NeuronCores (8 samples each). Per core:
  Phase 1: choice features cf = relu(l1(memory)) computed once into SBUF in a
           transposed layout cfT[h, (t, b, k)] (fp32, ~12.6 MB).
  Scan:    511 fully-unrolled steps, run as TWO interleaved independent chains
           (samples 0-3 / 4-7) so the engines overlap the chains' serial
           latencies. Everything lives transposed (feature dim on partitions,
           (b,k) on the free axis); per step and half:
             hid = relu(P1cf @ cf_t + P1h @ h + b)          (PE x2 + ACT relu)
             raw = p2 @ hid                                  (PE -> [1,24])
             noisy = max(raw, -p2b) + (gumbel+maskneg+p2b)   (DVE stt, staged out)
             onehot = (noisy >= rowmax)                      (DVE reduce + cmp)
             chosen = sum_k onehot * cf_t                    (PE bcast + DVE x2)
             gates  = bias + Wcomb @ h + Wcf @ chosen        (PE, gates transposed)
             LSTM cell                                       (ACT sigmoid/tanh + DVE)
           Per-step rows (noisy scores, onehot) are staged in [1, 8*48] blocks
           and DMA'd every 8 steps into rotated [128, 4*48] buffers.
  End:     logits = noisy - gumbel; batched log-softmax gather (exp/ln on ACT)
           + masked sum -> log_probs; argmax decoded from onehot -> idx.
Host side only reshapes/transposes inputs and bakes gumbel+mask+bias tensors.
"""

import os
import sys

import numpy as np

for _p in ("/opt/trn_rl_repo", "/root/.axon_site/_ro/trn_rl_repo"):
    if os.path.isdir(_p) and _p not in sys.path:
        sys.path.append(_p)

B, L, E, H, C = 64, 512, 256, 128, 128
T = L - 1  # 511
NCORES = 8
BL = B // NCORES  # 8 samples per core
K6 = 6
BK = BL * K6  # 48
HB = BL // 2  # 4 samples per half-chain
HK = HB * K6  # 24
NEG = np.float32(-1.0e30)


def _build_program(p2b_val: float, T_: int):
    import concourse.bass as bass
    import concourse.tile as tile
    from concourse import bacc, mybir

    f32 = mybir.dt.float32
    i32 = mybir.dt.int32
    AL = mybir.AluOpType
    AF = mybir.ActivationFunctionType
    AX = mybir.AxisListType

    NT_ = (T_ + 127) // 128  # rotated-layout column blocks

    nc = bacc.Bacc(
        "TRN2", target_bir_lowering=False, debug=False, num_devices=NCORES
    )

    d_memT = nc.dram_tensor("memT", [E, BL * T_], f32, kind="ExternalInput")
    d_gp2 = nc.dram_tensor("gp2", [1, T_ * BK], f32, kind="ExternalInput")
    d_gmr = nc.dram_tensor("gmr", [128, NT_ * BK], f32, kind="ExternalInput")
    d_mb = nc.dram_tensor("mb", [128, NT_ * BL], f32, kind="ExternalInput")
    d_kp = nc.dram_tensor("kp", [128, NT_ * BK], f32, kind="ExternalInput")
    d_l1wT = nc.dram_tensor("l1wT", [E, 6 * H], f32, kind="ExternalInput")
    d_l1b6 = nc.dram_tensor("l1b6", [128, 6], f32, kind="ExternalInput")
    d_p1cfT = nc.dram_tensor("p1cfT", [128, 128], f32, kind="ExternalInput")
    d_p1hT = nc.dram_tensor("p1hT", [128, 128], f32, kind="ExternalInput")
    d_p1b = nc.dram_tensor("p1b", [128, 1], f32, kind="ExternalInput")
    d_p2wT = nc.dram_tensor("p2wT", [128, 1], f32, kind="ExternalInput")
    d_wcfT = nc.dram_tensor("wcfT", [128, 4 * C], f32, kind="ExternalInput")
    d_wcoT = nc.dram_tensor("wcoT", [128, 4 * C], f32, kind="ExternalInput")
    d_bcr = nc.dram_tensor("bcr", [4, C], f32, kind="ExternalInput")
    d_bcT = nc.dram_tensor("bcT", [C, 4], f32, kind="ExternalInput")
    d_e46 = nc.dram_tensor("e46", [4, 4 * BK], f32, kind="ExternalInput")
    d_id128 = nc.dram_tensor("id128", [128, 128], f32, kind="ExternalInput")
    d_oidx = nc.dram_tensor("out_idx", [BL, T_], i32, kind="ExternalOutput")
    d_olp = nc.dram_tensor("out_lp", [BL, 1], f32, kind="ExternalOutput")

    with tile.TileContext(nc) as tc:
        with (
            tc.tile_pool(name="persist", bufs=1) as P,
            tc.tile_pool(name="blk", bufs=3) as BP,
            tc.tile_pool(name="work", bufs=4) as W,
        ):
            # ---- persistent SBUF tiles
            l1b6 = P.tile([128, 6], f32, tag="l1b6")
            cft = P.tile([128, T_, BK], f32, tag="cft")
            p1cf = P.tile([128, 128], f32, tag="p1cf")
            p1h = P.tile([128, 128], f32, tag="p1h")
            p1b = P.tile([128, 1], f32, tag="p1b")
            p2w = P.tile([128, 1], f32, tag="p2w")
            wcf = P.tile([128, 4 * C], f32, tag="wcf")
            wco = P.tile([128, 4 * C], f32, tag="wco")
            bcr4 = P.tile([4, C], f32, tag="bcr4")
            bcT = P.tile([C, 4], f32, tag="bcT")
            e46 = P.tile([4, 4 * BK], f32, tag="e46")
            id128 = P.tile([128, 128], f32, tag="id128")
            gmr = P.tile([128, NT_ * BK], f32, tag="gmr")
            mbs = P.tile([128, NT_ * BL], f32, tag="mbs")
            kps = P.tile([128, NT_ * BK], f32, tag="kps")
            scb = P.tile([128, NT_, BK], f32, tag="scb")
            oneb = P.tile([128, NT_, BK], f32, tag="oneb")

            ones1128 = P.tile([1, 128], f32, tag="ones1128")
            ones128 = P.tile([128, 1], f32, tag="ones128")
            zero128 = P.tile([128, 1], f32, tag="zero128")

            # ---- input DMAs
            nc.sync.dma_start(out=l1b6[:], in_=d_l1b6.ap())
            nc.sync.dma_start(out=p1cf[:], in_=d_p1cfT.ap())
            nc.sync.dma_start(out=p1h[:], in_=d_p1hT.ap())
            nc.sync.dma_start(out=p1b[:], in_=d_p1b.ap())
            nc.sync.dma_start(out=p2w[:], in_=d_p2wT.ap())
            nc.sync.dma_start(out=wcf[:], in_=d_wcfT.ap())
            nc.sync.dma_start(out=wco[:], in_=d_wcoT.ap())
            nc.sync.dma_start(out=bcr4[:], in_=d_bcr.ap())
            nc.sync.dma_start(out=bcT[:], in_=d_bcT.ap())
            nc.sync.dma_start(out=e46[:], in_=d_e46.ap())
            nc.sync.dma_start(out=id128[:], in_=d_id128.ap())
            nc.sync.dma_start(out=gmr[:], in_=d_gmr.ap())
            nc.sync.dma_start(out=mbs[:], in_=d_mb.ap())
            nc.sync.dma_start(out=kps[:], in_=d_kp.ap())


            nc.vector.memset(ones1128[:], 1.0)
            nc.vector.memset(ones128[:], 1.0)
            nc.vector.memset(zero128[:], 0.0)
            nc.vector.memset(scb[:], 0.0)
            nc.vector.memset(oneb[:], 0.0)

            # ---- phase 1: cf = relu(l1 @ mem + b), scattered into cfT layout
            with (
                tc.tile_pool(name="ph1sbuf", bufs=1) as P1S,
                tc.tile_pool(name="ph1psum", bufs=4, space="PSUM") as PP1,
            ):
                memT0 = P1S.tile([128, BL * T_], f32, tag="memT0")
                memT1 = P1S.tile([128, BL * T_], f32, tag="memT1")
                l1w0 = P1S.tile([128, 6 * H], f32, tag="l1w0")
                l1w1 = P1S.tile([128, 6 * H], f32, tag="l1w1")
                nc.sync.dma_start(out=memT0[:], in_=d_memT.ap()[0:128, :])
                nc.sync.dma_start(out=memT1[:], in_=d_memT.ap()[128:256, :])
                nc.sync.dma_start(out=l1w0[:], in_=d_l1wT.ap()[0:128, :])
                nc.sync.dma_start(out=l1w1[:], in_=d_l1wT.ap()[128:256, :])
                for b in range(BL):
                    for k in range(K6):
                        pC = PP1.tile([128, T_], f32, tag="pC")
                        nc.tensor.matmul(
                            pC[:],
                            l1w0[:, k * H : (k + 1) * H],
                            memT0[:, b * T_ : (b + 1) * T_],
                            start=True,
                            stop=False,
                        )
                        nc.tensor.matmul(
                            pC[:],
                            l1w1[:, k * H : (k + 1) * H],
                            memT1[:, b * T_ : (b + 1) * T_],
                            start=False,
                            stop=True,
                        )
                        nc.vector.tensor_scalar(
                            cft[:, :, b * K6 + k],
                            pC[:],
                            l1b6[:, k : k + 1],
                            0.0,
                            AL.add,
                            AL.max,
                        )

            # ---- scan: full-width chain; speculative LSTM over all 6 candidates
            # off-chain: gates_k = bias + Wco@h + (Wcf@cf_k pre-batched); LSTM for
            # all k; chain only: hid-relu -> scores -> noisy -> argmax -> select.
            MB = 16  # steps per staging block
            with tc.tile_pool(name="spsum", bufs=2, space="PSUM") as SP:
                hsel = W.tile([128, 2 * BL], f32, tag="hcsel", name="hsel0")
                nc.vector.memset(hsel[:], 0.0)
                csel0 = W.tile([128, BL], f32, tag="csel0", name="csel0")
                nc.vector.memset(csel0[:], 0.0)
                csel = csel0
                nb8 = None
                one16 = None
                gpblk = None
                gcf16 = None
                for t in range(T_):
                    m16 = t % MB
                    if m16 == 0:
                        nb = min(MB, T_ - t)
                        gpblk = BP.tile([1, MB * BK], f32, tag="gpblk")
                        nc.sync.dma_start(
                            out=gpblk[:, : nb * BK],
                            in_=d_gp2.ap()[:, t * BK : (t + nb) * BK],
                        )
                        nb8 = BP.tile([1, MB * BK], f32, tag="nb8")
                        one16 = BP.tile([1, MB * BK], f32, tag="one16")
                        # pre-batch Wcf @ cf for the block: gcf16[t', (g,b,k)]
                        gcf16 = BP.tile([128, MB, 4 * BK], f32, tag="gcf16", bufs=2)
                        for g in range(4):
                            for c2 in range(2):
                                t0c = c2 * 8
                                ch = min(8, nb - t0c)
                                if ch <= 0:
                                    continue
                                psC = SP.tile(
                                    [128, 8 * BK], f32, tag="psC", bufs=1, name="psC"
                                )
                                nc.tensor.matmul(
                                    psC[:, : ch * BK],
                                    wcf[:, g * C : (g + 1) * C],
                                    cft[:, t + t0c : t + t0c + ch, :].rearrange(
                                        "p a k -> p (a k)"
                                    ),
                                    start=True,
                                    stop=True,
                                )
                                nc.scalar.activation(
                                    gcf16[
                                        :, t0c : t0c + ch, g * BK : (g + 1) * BK
                                    ],
                                    psC[:, : ch * BK].rearrange(
                                        "p (a k) -> p a k", k=BK
                                    ),
                                    AF.Identity,
                                    bias=bcT[:, g : g + 1],
                                    scale=1.0,
                                )

                    cft_t = cft[:, t, :]  # [128, 48]
                    hT48b = (
                        hsel[:, 0:BL].unsqueeze(2).to_broadcast([128, BL, K6])
                    )
                    hT8 = hsel[:, 0:BL]

                    # hid pre-activation (chain) + gate psums (off-chain)
                    pA = SP.tile([128, BK], f32, tag="pA", bufs=2)
                    nc.tensor.matmul(pA[:], p1cf[:], cft_t, start=True, stop=False)
                    nc.tensor.matmul(
                        pA.rearrange("p (b k) -> p b k", k=K6),
                        p1h[:],
                        hT48b,
                        start=False,
                        stop=True,
                    )

                    pG48 = SP.tile([128, 4 * BK], f32, tag="pG48", bufs=2)
                    nc.tensor.matmul(
                        pG48[:], id128[:], gcf16[:, m16, :], start=True, stop=False
                    )
                    hT8b = hT8.unsqueeze(2).to_broadcast([128, BL, K6])
                    for g in range(4):
                        nc.tensor.matmul(
                            pG48[:, g * BK : (g + 1) * BK].rearrange(
                                "p (b k) -> p b k", k=K6
                            ),
                            wco[:, g * C : (g + 1) * C],
                            hT8b,
                            start=False,
                            stop=(g == 3),
                        )

                    # speculative LSTM for all 6 candidates (off-chain)
                    # i/f/o weights pre-scaled 0.5 on host: sigmoid(x) =
                    # sigmoid-table with scale=2 on the halved gates.
                    sig = W.tile([128, 3 * BK], f32, tag="sig")
                    nc.scalar.activation(
                        sig[:], pG48[:, 0 : 3 * BK], AF.Sigmoid, bias=zero128[:], scale=2.0
                    )
                    th = W.tile([128, 4 * BK], f32, tag="th")
                    nc.scalar.activation(
                        th[:, 3 * BK : 4 * BK],
                        pG48[:, 3 * BK : 4 * BK],
                        AF.Tanh,
                        bias=zero128[:],
                        scale=1.0,
                    )
                    hc_cand = W.tile([128, 2 * BK], f32, tag="hc_cand")
                    t1 = W.tile([128, BK], f32, tag="t1")
                    nc.vector.tensor_mul(
                        t1.rearrange("p (b k) -> p b k", k=K6),
                        sig[:, BK : 2 * BK].rearrange("p (b k) -> p b k", k=K6),
                        csel.unsqueeze(2).to_broadcast([128, BL, K6]),
                    )
                    t2 = W.tile([128, BK], f32, tag="t2")
                    nc.vector.tensor_mul(t2[:], sig[:, 0:BK], th[:, 3 * BK : 4 * BK])
                    nc.vector.tensor_add(hc_cand[:, BK : 2 * BK], t1[:], t2[:])
                    tcs = W.tile([128, BK], f32, tag="tcs")
                    nc.scalar.activation(
                        tcs[:], hc_cand[:, BK : 2 * BK], AF.Tanh, bias=zero128[:], scale=1.0
                    )
                    nc.vector.tensor_mul(
                        hc_cand[:, 0:BK], sig[:, 2 * BK : 3 * BK], tcs[:]
                    )

                    # chain: hid -> scores -> noisy -> argmax -> select h,c
                    hid = W.tile([128, BK], f32, tag="hid")
                    nc.vector.tensor_scalar(
                        hid[:], pA[:], p1b[:], 0.0, AL.add, AL.max
                    )
                    pS = SP.tile([1, BK], f32, tag="pS", bufs=2)
                    nc.tensor.matmul(pS[:], p2w[:], hid[:], start=True, stop=True)
                    ms = slice(m16 * BK, (m16 + 1) * BK)
                    nc.vector.scalar_tensor_tensor(
                        nb8[:, ms], pS[:], -p2b_val, gpblk[:, ms], AL.max, AL.add
                    )
                    nv = nb8[:, ms].rearrange("p (b k) -> p b k", k=K6)
                    nmax = W.tile([1, BL], f32, tag="nmax")
                    nc.vector.reduce_max(nmax[:], nv, axis=AX.X)
                    one_v = one16[:, ms].rearrange("p (b k) -> p b k", k=K6)
                    nmax_b = nmax.unsqueeze(2).to_broadcast([1, BL, K6])
                    nc.vector.tensor_tensor(one_v, nv, nmax_b, op=AL.is_ge)
                    pO2 = SP.tile([128, 2 * BK], f32, tag="pO2", bufs=1)
                    nc.tensor.matmul(
                        pO2[:],
                        ones1128[:],
                        one16[:, ms]
                        .rearrange("p (a k) -> p a k", a=1)
                        .to_broadcast([1, 2, BK]),
                        start=True,
                        stop=True,
                    )
                    tmp96 = W.tile([128, 2 * BK], f32, tag="tmp96")
                    nc.vector.tensor_mul(tmp96[:], hc_cand[:], pO2[:])
                    hcsel = W.tile([128, 2 * BL], f32, tag="hcsel")
                    nc.vector.reduce_sum(
                        hcsel[:],
                        tmp96.rearrange("p (a b k) -> p a b k", a=2, k=K6),
                        axis=AX.X,
                    )
                    hsel = hcsel
                    csel = hcsel[:, BL : 2 * BL]

                    if m16 == MB - 1 or t == T_ - 1:
                        nb = m16 + 1
                        p0 = (t - m16) % 128
                        jj = (t - m16) // 128
                        nc.sync.dma_start(
                            out=scb[p0 : p0 + nb, jj, :],
                            in_=nb8[:, : nb * BK].rearrange(
                                "p (n k) -> p n k", k=BK
                            ),
                        )
                        nc.sync.dma_start(
                            out=oneb[p0 : p0 + nb, jj, :],
                            in_=one16[:, : nb * BK].rearrange(
                                "p (n k) -> p n k", k=BK
                            ),
                        )

            # ---- end phase: log-probs and indices
            with (
                tc.tile_pool(name="endp", bufs=1) as EP,
                tc.tile_pool(name="endpsum", bufs=1, space="PSUM") as PEP,
            ):
                scb_f = scb.rearrange("p j k -> p (j k)")
                oneb_f = oneb.rearrange("p j k -> p (j k)")
                logits = EP.tile([128, NT_ * BK], f32, tag="logits")
                nc.vector.tensor_sub(logits[:], scb_f, gmr[:])
                expv = EP.tile([128, NT_ * BK], f32, tag="expv")
                nc.scalar.activation(
                    expv[:], logits[:], AF.Exp, bias=zero128[:], scale=1.0
                )
                esum = EP.tile([128, NT_ * BL], f32, tag="esum")
                nc.vector.reduce_sum(
                    esum[:],
                    expv.rearrange("p (j b k) -> p j b k", b=BL, k=K6),
                    axis=AX.X,
                )
                lse = EP.tile([128, NT_ * BL], f32, tag="lse")
                nc.scalar.activation(
                    lse[:], esum[:], AF.Ln, bias=zero128[:], scale=1.0
                )
                selp = EP.tile([128, NT_ * BK], f32, tag="selp")
                nc.vector.tensor_mul(selp[:], logits[:], oneb_f)
                sel = EP.tile([128, NT_ * BL], f32, tag="sel")
                nc.vector.reduce_sum(
                    sel[:],
                    selp.rearrange("p (j b k) -> p j b k", b=BL, k=K6),
                    axis=AX.X,
                )
                diff = EP.tile([128, NT_ * BL], f32, tag="diff")
                nc.vector.tensor_sub(diff[:], sel[:], lse[:])
                masked = EP.tile([128, NT_ * BL], f32, tag="masked")
                nc.vector.tensor_mul(masked[:], diff[:], mbs[:])
                part = EP.tile([128, BL], f32, tag="part")
                nc.vector.reduce_sum(
                    part[:],
                    masked.rearrange("p (j b) -> p b j", b=BL),
                    axis=AX.X,
                )
                accp = PEP.tile([BL, 1], f32, tag="accp")
                nc.tensor.matmul(accp[:], part[:], ones128[:], start=True, stop=True)
                lp_sb = EP.tile([BL, 1], f32, tag="lp_sb")
                nc.vector.tensor_copy(lp_sb[:], accp[:])
                nc.sync.dma_start(out=d_olp.ap(), in_=lp_sb[:])

                ksel = EP.tile([128, NT_ * BK], f32, tag="ksel")
                nc.vector.tensor_mul(ksel[:], oneb_f, kps[:])
                idxf = EP.tile([128, NT_ * BL], f32, tag="idxf")
                nc.vector.reduce_sum(
                    idxf[:],
                    ksel.rearrange("p (j b k) -> p j b k", b=BL, k=K6),
                    axis=AX.X,
                )
                idxi = EP.tile([128, NT_ * BL], i32, tag="idxi")
                nc.vector.tensor_copy(idxi[:], idxf[:])
                for j in range(NT_):
                    pmax = min(128, T_ - j * 128)
                    nc.sync.dma_start(
                        out=d_oidx.ap()[:, j * 128 : j * 128 + pmax].transpose(
                            [1, 0]
                        ),
                        in_=idxi[0:pmax, j * BL : (j + 1) * BL],
                    )

    nc.compile()
    return nc


def _prep_shared(inputs, T_):
    """Host-side weight prep (shared across cores)."""
    f = np.float32
    l1_w = inputs["l1_w"].astype(f)
    p1_w = inputs["p1_w"].astype(f)
    w_ih = inputs["w_ih"].astype(f)
    w_hh = inputs["w_hh"].astype(f)
    bc = (inputs["b_ih"] + inputs["b_hh"]).astype(f)
    NT_ = (T_ + 127) // 128
    gorder = [0, 1, 3, 2]  # i, f, o, g
    wcfT = np.empty((128, 4 * C), f)
    wcoT = np.empty((128, 4 * C), f)
    bcr = np.empty((4, C), f)
    for gi, g in enumerate(gorder):
        rows = slice(g * C, (g + 1) * C)
        s5 = f(0.5) if gi < 3 else f(1.0)  # sigmoid-as-tanh pre-scale (i,f,o)
        wcfT[:, gi * C : (gi + 1) * C] = s5 * w_ih[rows, :H].T
        wcoT[:, gi * C : (gi + 1) * C] = s5 * (w_ih[rows, H:] + w_hh[rows, :]).T
        bcr[gi, :] = s5 * bc[rows]
    kcol = np.tile(np.arange(K6, dtype=f), BL)  # [48]
    kp = np.broadcast_to(kcol, (128, NT_, BK)).reshape(128, NT_ * BK).copy()
    bcT = bcr.T.copy()
    return {
        "bcT": bcT,
        "l1wT": np.ascontiguousarray(l1_w.T),
        "l1b6": np.ascontiguousarray(inputs["l1_b"].astype(f).reshape(6, 128).T),
        "p1cfT": np.ascontiguousarray(p1_w[:, :H].T),
        "p1hT": np.ascontiguousarray(p1_w[:, H:].T),
        "p1b": inputs["p1_b"].astype(f).reshape(128, 1).copy(),
        "p2wT": np.ascontiguousarray(inputs["p2_w"].astype(f).T),
        "wcfT": wcfT,
        "wcoT": wcoT,
        "bcr": bcr,
        "e46": np.kron(np.eye(4, dtype=f), np.ones((1, BK), f)),
        "id128": np.eye(128, dtype=f),
        "kp": kp,
    }


def _rot(x_t, T_, width):
    """[T, width] -> rotated [128, NT*width] (partition = t%128, block = t//128)."""
    f = np.float32
    NT_ = (T_ + 127) // 128
    Xp = np.zeros((NT_ * 128, width), f)
    Xp[:T_] = x_t
    return np.ascontiguousarray(
        Xp.reshape(NT_, 128, width).transpose(1, 0, 2).reshape(128, NT_ * width)
    )


def _prep_core(inputs, ci, T_):
    f = np.float32
    p2b = f(np.asarray(inputs["p2_b"]).reshape(-1)[0])
    sl = slice(ci * BL, (ci + 1) * BL)
    mem = inputs["memory"][sl, 2 : 2 + T_, :].astype(f)  # [8, T, 256]
    memT = np.ascontiguousarray(mem.transpose(2, 0, 1).reshape(E, BL * T_))
    mask_t = inputs["mask"][sl, 1 : 1 + T_, :]  # [8, T, 6]
    SMt = np.where(mask_t, f(0.0), NEG).astype(f).transpose(1, 0, 2)  # [T, 8, 6]
    gum = inputs["gumbel"][:T_, sl, :].astype(f)  # [T, 8, 6]
    gp2 = (gum + SMt + p2b).reshape(1, T_ * BK).astype(f)
    gmr = _rot(gum.reshape(T_, BK), T_, BK)
    length = inputs["length"][sl].astype(np.int64)
    Mt = (length[None, :] > (np.arange(T_) + 1)[:, None]).astype(f)  # [T, 8]
    mb = _rot(Mt, T_, BL)
    return {"memT": memT, "gp2": gp2, "gmr": gmr, "mb": mb}


def _make_in_maps(inputs, T_):
    shared = _prep_shared(inputs, T_)
    return [dict(shared, **_prep_core(inputs, ci, T_)) for ci in range(NCORES)]


LAST_RESULTS = None


def kernel(**inputs):
    global LAST_RESULTS
    from concourse.bass_utils import run_bass_kernel_spmd

    inputs = {k: np.asarray(v) for k, v in inputs.items()}
    in_maps = _make_in_maps(inputs, T)
    nc = _build_program(float(np.asarray(inputs["p2_b"]).reshape(-1)[0]), T)
    res = run_bass_kernel_spmd(nc, in_maps, core_ids=list(range(NCORES)))
    LAST_RESULTS = res
    idx = np.concatenate([r["out_idx"] for r in res.results], axis=0)
    lp = np.concatenate([r["out_lp"][:, 0] for r in res.results], axis=0)
    return idx.astype(np.int32), lp.astype(np.float32)


# revision 45
# speedup vs baseline: 1.0836x; 1.0123x over previous
"""Trainium2 Bass kernel for the Actor sampling module (nn_Actor_47588237640247).

Strategy: pure data-parallel across 8 NeuronCores (8 samples each). Per core:
  Phase 1: choice features cf = relu(l1(memory)) computed once into SBUF in a
           transposed layout cfT[h, (t, b, k)] (fp32, ~12.6 MB).
  Scan:    511 fully-unrolled steps, run as TWO interleaved independent chains
           (samples 0-3 / 4-7) so the engines overlap the chains' serial
           latencies. Everything lives transposed (feature dim on partitions,
           (b,k) on the free axis); per step and half:
             hid = relu(P1cf @ cf_t + P1h @ h + b)          (PE x2 + ACT relu)
             raw = p2 @ hid                                  (PE -> [1,24])
             noisy = max(raw, -p2b) + (gumbel+maskneg+p2b)   (DVE stt, staged out)
             onehot = (noisy >= rowmax)                      (DVE reduce + cmp)
             chosen = sum_k onehot * cf_t                    (PE bcast + DVE x2)
             gates  = bias + Wcomb @ h + Wcf @ chosen        (PE, gates transposed)
             LSTM cell                                       (ACT sigmoid/tanh + DVE)
           Per-step rows (noisy scores, onehot) are staged in [1, 8*48] blocks
           and DMA'd every 8 steps into rotated [128, 4*48] buffers.
  End:     logits = noisy - gumbel; batched log-softmax gather (exp/ln on ACT)
           + masked sum -> log_probs; argmax decoded from onehot -> idx.
Host side only reshapes/transposes inputs and bakes gumbel+mask+bias tensors.
"""

import os
import sys

import numpy as np

for _p in ("/opt/trn_rl_repo", "/root/.axon_site/_ro/trn_rl_repo"):
    if os.path.isdir(_p) and _p not in sys.path:
        sys.path.append(_p)

B, L, E, H, C = 64, 512, 256, 128, 128
T = L - 1  # 511
NCORES = 8
BL = B // NCORES  # 8 samples per core
K6 = 6
BK = BL * K6  # 48
HB = BL // 2  # 4 samples per half-chain
HK = HB * K6  # 24
NEG = np.float32(-1.0e30)


def _build_program(p2b_val: float, T_: int):
    import concourse.bass as bass
    import concourse.tile as tile
    from concourse import bacc, mybir

    f32 = mybir.dt.float32
    i32 = mybir.dt.int32
    AL = mybir.AluOpType
    AF = mybir.ActivationFunctionType
    AX = mybir.AxisListType

    NT_ = (T_ + 127) // 128  # rotated-layout column blocks

    nc = bacc.Bacc(
        "TRN2", target_bir_lowering=False, debug=False, num_devices=NCORES
    )

    d_memT = nc.dram_tensor("memT", [E, BL * T_], f32, kind="ExternalInput")
    d_gp2 = nc.dram_tensor("gp2", [1, T_ * BK], f32, kind="ExternalInput")
    d_gmr = nc.dram_tensor("gmr", [128, NT_ * BK], f32, kind="ExternalInput")
    d_mb = nc.dram_tensor("mb", [128, NT_ * BL], f32, kind="ExternalInput")
    d_kp = nc.dram_tensor("kp", [128, NT_ * BK], f32, kind="ExternalInput")
    d_l1wT = nc.dram_tensor("l1wT", [E, 6 * H], f32, kind="ExternalInput")
    d_l1b6 = nc.dram_tensor("l1b6", [128, 6], f32, kind="ExternalInput")
    d_p1cfT = nc.dram_tensor("p1cfT", [128, 128], f32, kind="ExternalInput")
    d_p1hT = nc.dram_tensor("p1hT", [128, 128], f32, kind="ExternalInput")
    d_p1b = nc.dram_tensor("p1b", [128, 1], f32, kind="ExternalInput")
    d_p2wT = nc.dram_tensor("p2wT", [128, 1], f32, kind="ExternalInput")
    d_wcfT = nc.dram_tensor("wcfT", [128, 4 * C], f32, kind="ExternalInput")
    d_wcoT = nc.dram_tensor("wcoT", [128, 4 * C], f32, kind="ExternalInput")
    d_bcr = nc.dram_tensor("bcr", [4, C], f32, kind="ExternalInput")
    d_bcT = nc.dram_tensor("bcT", [C, 4], f32, kind="ExternalInput")
    d_e46 = nc.dram_tensor("e46", [4, 4 * BK], f32, kind="ExternalInput")
    d_id128 = nc.dram_tensor("id128", [128, 128], f32, kind="ExternalInput")
    d_oidx = nc.dram_tensor("out_idx", [BL, T_], i32, kind="ExternalOutput")
    d_olp = nc.dram_tensor("out_lp", [BL, 1], f32, kind="ExternalOutput")

    with tile.TileContext(nc) as tc:
        with (
            tc.tile_pool(name="persist", bufs=1) as P,
            tc.tile_pool(name="blk", bufs=3) as BP,
            tc.tile_pool(name="work", bufs=4) as W,
        ):
            # ---- persistent SBUF tiles
            l1b6 = P.tile([128, 6], f32, tag="l1b6")
            cft = P.tile([128, T_, BK], f32, tag="cft")
            p1cf = P.tile([128, 128], f32, tag="p1cf")
            p1h = P.tile([128, 128], f32, tag="p1h")
            p1b = P.tile([128, 1], f32, tag="p1b")
            p2w = P.tile([128, 1], f32, tag="p2w")
            wcf = P.tile([128, 4 * C], f32, tag="wcf")
            wco = P.tile([128, 4 * C], f32, tag="wco")
            bcr4 = P.tile([4, C], f32, tag="bcr4")
            bcT = P.tile([C, 4], f32, tag="bcT")
            e46 = P.tile([4, 4 * BK], f32, tag="e46")
            id128 = P.tile([128, 128], f32, tag="id128")
            gmr = P.tile([128, NT_ * BK], f32, tag="gmr")
            mbs = P.tile([128, NT_ * BL], f32, tag="mbs")
            kps = P.tile([128, NT_ * BK], f32, tag="kps")
            scb = P.tile([128, NT_, BK], f32, tag="scb")
            oneb = P.tile([128, NT_, BK], f32, tag="oneb")

            ones1128 = P.tile([1, 128], f32, tag="ones1128")
            ones128 = P.tile([128, 1], f32, tag="ones128")
            zero128 = P.tile([128, 1], f32, tag="zero128")

            # ---- input DMAs
            nc.sync.dma_start(out=l1b6[:], in_=d_l1b6.ap())
            nc.sync.dma_start(out=p1cf[:], in_=d_p1cfT.ap())
            nc.sync.dma_start(out=p1h[:], in_=d_p1hT.ap())
            nc.sync.dma_start(out=p1b[:], in_=d_p1b.ap())
            nc.sync.dma_start(out=p2w[:], in_=d_p2wT.ap())
            nc.sync.dma_start(out=wcf[:], in_=d_wcfT.ap())
            nc.sync.dma_start(out=wco[:], in_=d_wcoT.ap())
            nc.sync.dma_start(out=bcr4[:], in_=d_bcr.ap())
            nc.sync.dma_start(out=bcT[:], in_=d_bcT.ap())
            nc.sync.dma_start(out=e46[:], in_=d_e46.ap())
            nc.sync.dma_start(out=id128[:], in_=d_id128.ap())
            nc.sync.dma_start(out=gmr[:], in_=d_gmr.ap())
            nc.sync.dma_start(out=mbs[:], in_=d_mb.ap())
            nc.sync.dma_start(out=kps[:], in_=d_kp.ap())


            nc.vector.memset(ones1128[:], 1.0)
            nc.vector.memset(ones128[:], 1.0)
            nc.vector.memset(zero128[:], 0.0)
            nc.vector.memset(scb[:], 0.0)
            nc.vector.memset(oneb[:], 0.0)

            # ---- phase 1: cf = relu(l1 @ mem + b), scattered into cfT layout
            with (
                tc.tile_pool(name="ph1sbuf", bufs=1) as P1S,
                tc.tile_pool(name="ph1psum", bufs=4, space="PSUM") as PP1,
            ):
                memT0 = P1S.tile([128, BL * T_], f32, tag="memT0")
                memT1 = P1S.tile([128, BL * T_], f32, tag="memT1")
                l1w0 = P1S.tile([128, 6 * H], f32, tag="l1w0")
                l1w1 = P1S.tile([128, 6 * H], f32, tag="l1w1")
                nc.sync.dma_start(out=memT0[:], in_=d_memT.ap()[0:128, :])
                nc.sync.dma_start(out=memT1[:], in_=d_memT.ap()[128:256, :])
                nc.sync.dma_start(out=l1w0[:], in_=d_l1wT.ap()[0:128, :])
                nc.sync.dma_start(out=l1w1[:], in_=d_l1wT.ap()[128:256, :])
                for b in range(BL):
                    for k in range(K6):
                        pC = PP1.tile([128, T_], f32, tag="pC")
                        nc.tensor.matmul(
                            pC[:],
                            l1w0[:, k * H : (k + 1) * H],
                            memT0[:, b * T_ : (b + 1) * T_],
                            start=True,
                            stop=False,
                        )
                        nc.tensor.matmul(
                            pC[:],
                            l1w1[:, k * H : (k + 1) * H],
                            memT1[:, b * T_ : (b + 1) * T_],
                            start=False,
                            stop=True,
                        )
                        nc.vector.tensor_scalar(
                            cft[:, :, b * K6 + k],
                            pC[:],
                            l1b6[:, k : k + 1],
                            0.0,
                            AL.add,
                            AL.max,
                        )

            # ---- scan: full-width chain; speculative LSTM over all 6 candidates
            # off-chain: gates_k = bias + Wco@h + (Wcf@cf_k pre-batched); LSTM for
            # all k; chain only: hid-relu -> scores -> noisy -> argmax -> select.
            MB = 16  # steps per staging block
            with tc.tile_pool(name="spsum", bufs=2, space="PSUM") as SP:
                hsel = W.tile([128, 2 * BL], f32, tag="hcsel", name="hsel0")
                nc.vector.memset(hsel[:], 0.0)
                csel0 = W.tile([128, BL], f32, tag="csel0", name="csel0")
                nc.vector.memset(csel0[:], 0.0)
                csel = csel0
                nb8 = None
                one16 = None
                gpblk = None
                gcf16 = None
                for t in range(T_):
                    m16 = t % MB
                    if m16 == 0:
                        nb = min(MB, T_ - t)
                        gpblk = BP.tile([1, MB * BK], f32, tag="gpblk")
                        nc.sync.dma_start(
                            out=gpblk[:, : nb * BK],
                            in_=d_gp2.ap()[:, t * BK : (t + nb) * BK],
                        )
                        nb8 = BP.tile([1, MB * BK], f32, tag="nb8")
                        one16 = BP.tile([1, MB * BK], f32, tag="one16")
                        # pre-batch Wcf @ cf for the block: gcf16[t', (g,b,k)]
                        gcf16 = BP.tile([128, MB, 4 * BK], f32, tag="gcf16", bufs=2)
                        for g in range(4):
                            for c2 in range(4):
                                t0c = c2 * 4
                                ch = min(4, nb - t0c)
                                if ch <= 0:
                                    continue
                                psC = SP.tile(
                                    [128, 8 * BK], f32, tag="psC", bufs=2, name="psC"
                                )
                                nc.tensor.matmul(
                                    psC[:, : ch * BK],
                                    wcf[:, g * C : (g + 1) * C],
                                    cft[:, t + t0c : t + t0c + ch, :].rearrange(
                                        "p a k -> p (a k)"
                                    ),
                                    start=True,
                                    stop=True,
                                )
                                nc.scalar.activation(
                                    gcf16[
                                        :, t0c : t0c + ch, g * BK : (g + 1) * BK
                                    ],
                                    psC[:, : ch * BK].rearrange(
                                        "p (a k) -> p a k", k=BK
                                    ),
                                    AF.Identity,
                                    bias=bcT[:, g : g + 1],
                                    scale=1.0,
                                )

                    cft_t = cft[:, t, :]  # [128, 48]
                    hT48b = (
                        hsel[:, 0:BL].unsqueeze(2).to_broadcast([128, BL, K6])
                    )
                    hT8 = hsel[:, 0:BL]

                    # hid pre-activation (chain) + gate psums (off-chain)
                    pA = SP.tile([128, BK], f32, tag="pA", bufs=2)
                    nc.tensor.matmul(pA[:], p1cf[:], cft_t, start=True, stop=False)
                    nc.tensor.matmul(
                        pA.rearrange("p (b k) -> p b k", k=K6),
                        p1h[:],
                        hT48b,
                        start=False,
                        stop=True,
                    )

                    pG48 = SP.tile([128, 4 * BK], f32, tag="pG48", bufs=2)
                    nc.tensor.matmul(
                        pG48[:], id128[:], gcf16[:, m16, :], start=True, stop=False
                    )
                    hT8b = hT8.unsqueeze(2).to_broadcast([128, BL, K6])
                    for g in range(4):
                        nc.tensor.matmul(
                            pG48[:, g * BK : (g + 1) * BK].rearrange(
                                "p (b k) -> p b k", k=K6
                            ),
                            wco[:, g * C : (g + 1) * C],
                            hT8b,
                            start=False,
                            stop=(g == 3),
                        )

                    # speculative LSTM for all 6 candidates (off-chain)
                    # i/f/o weights pre-scaled 0.5 on host: sigmoid(x) =
                    # sigmoid-table with scale=2 on the halved gates.
                    sig = W.tile([128, 3 * BK], f32, tag="sig")
                    nc.scalar.activation(
                        sig[:], pG48[:, 0 : 3 * BK], AF.Sigmoid, bias=zero128[:], scale=2.0
                    )
                    th = W.tile([128, 4 * BK], f32, tag="th")
                    nc.scalar.activation(
                        th[:, 3 * BK : 4 * BK],
                        pG48[:, 3 * BK : 4 * BK],
                        AF.Tanh,
                        bias=zero128[:],
                        scale=1.0,
                    )
                    hc_cand = W.tile([128, 2 * BK], f32, tag="hc_cand")
                    t1 = W.tile([128, BK], f32, tag="t1")
                    nc.vector.tensor_mul(
                        t1.rearrange("p (b k) -> p b k", k=K6),
                        sig[:, BK : 2 * BK].rearrange("p (b k) -> p b k", k=K6),
                        csel.unsqueeze(2).to_broadcast([128, BL, K6]),
                    )
                    t2 = W.tile([128, BK], f32, tag="t2")
                    nc.vector.tensor_mul(t2[:], sig[:, 0:BK], th[:, 3 * BK : 4 * BK])
                    nc.vector.tensor_add(hc_cand[:, BK : 2 * BK], t1[:], t2[:])
                    tcs = W.tile([128, BK], f32, tag="tcs")
                    nc.scalar.activation(
                        tcs[:], hc_cand[:, BK : 2 * BK], AF.Tanh, bias=zero128[:], scale=1.0
                    )
                    nc.vector.tensor_mul(
                        hc_cand[:, 0:BK], sig[:, 2 * BK : 3 * BK], tcs[:]
                    )

                    # chain: hid -> scores -> noisy -> argmax -> select h,c
                    hid = W.tile([128, BK], f32, tag="hid")
                    nc.vector.tensor_scalar(
                        hid[:], pA[:], p1b[:], 0.0, AL.add, AL.max
                    )
                    pS = SP.tile([1, BK], f32, tag="pS", bufs=1)
                    nc.tensor.matmul(pS[:], p2w[:], hid[:], start=True, stop=True)
                    ms = slice(m16 * BK, (m16 + 1) * BK)
                    nc.vector.scalar_tensor_tensor(
                        nb8[:, ms], pS[:], -p2b_val, gpblk[:, ms], AL.max, AL.add
                    )
                    nv = nb8[:, ms].rearrange("p (b k) -> p b k", k=K6)
                    nmax = W.tile([1, BL], f32, tag="nmax")
                    nc.vector.reduce_max(nmax[:], nv, axis=AX.X)
                    one_v = one16[:, ms].rearrange("p (b k) -> p b k", k=K6)
                    nmax_b = nmax.unsqueeze(2).to_broadcast([1, BL, K6])
                    nc.vector.tensor_tensor(one_v, nv, nmax_b, op=AL.is_ge)
                    pO2 = SP.tile([128, 2 * BK], f32, tag="pO2", bufs=1)
                    nc.tensor.matmul(
                        pO2[:],
                        ones1128[:],
                        one16[:, ms]
                        .rearrange("p (a k) -> p a k", a=1)
                        .to_broadcast([1, 2, BK]),
                        start=True,
                        stop=True,
                    )
                    tmp96 = W.tile([128, 2 * BK], f32, tag="tmp96")
                    nc.vector.tensor_mul(tmp96[:], hc_cand[:], pO2[:])
                    hcsel = W.tile([128, 2 * BL], f32, tag="hcsel")
                    nc.vector.reduce_sum(
                        hcsel[:],
                        tmp96.rearrange("p (a b k) -> p a b k", a=2, k=K6),
                        axis=AX.X,
                    )
                    hsel = hcsel
                    csel = hcsel[:, BL : 2 * BL]

                    if m16 == MB - 1 or t == T_ - 1:
                        nb = m16 + 1
                        p0 = (t - m16) % 128
                        jj = (t - m16) // 128
                        nc.sync.dma_start(
                            out=scb[p0 : p0 + nb, jj, :],
                            in_=nb8[:, : nb * BK].rearrange(
                                "p (n k) -> p n k", k=BK
                            ),
                        )
                        nc.sync.dma_start(
                            out=oneb[p0 : p0 + nb, jj, :],
                            in_=one16[:, : nb * BK].rearrange(
                                "p (n k) -> p n k", k=BK
                            ),
                        )

            # ---- end phase: log-probs and indices
            with (
                tc.tile_pool(name="endp", bufs=1) as EP,
                tc.tile_pool(name="endpsum", bufs=1, space="PSUM") as PEP,
            ):
                scb_f = scb.rearrange("p j k -> p (j k)")
                oneb_f = oneb.rearrange("p j k -> p (j k)")
                logits = EP.tile([128, NT_ * BK], f32, tag="logits")
                nc.vector.tensor_sub(logits[:], scb_f, gmr[:])
                expv = EP.tile([128, NT_ * BK], f32, tag="expv")
                nc.scalar.activation(
                    expv[:], logits[:], AF.Exp, bias=zero128[:], scale=1.0
                )
                esum = EP.tile([128, NT_ * BL], f32, tag="esum")
                nc.vector.reduce_sum(
                    esum[:],
                    expv.rearrange("p (j b k) -> p j b k", b=BL, k=K6),
                    axis=AX.X,
                )
                lse = EP.tile([128, NT_ * BL], f32, tag="lse")
                nc.scalar.activation(
                    lse[:], esum[:], AF.Ln, bias=zero128[:], scale=1.0
                )
                selp = EP.tile([128, NT_ * BK], f32, tag="selp")
                nc.vector.tensor_mul(selp[:], logits[:], oneb_f)
                sel = EP.tile([128, NT_ * BL], f32, tag="sel")
                nc.vector.reduce_sum(
                    sel[:],
                    selp.rearrange("p (j b k) -> p j b k", b=BL, k=K6),
                    axis=AX.X,
                )
                diff = EP.tile([128, NT_ * BL], f32, tag="diff")
                nc.vector.tensor_sub(diff[:], sel[:], lse[:])
                masked = EP.tile([128, NT_ * BL], f32, tag="masked")
                nc.vector.tensor_mul(masked[:], diff[:], mbs[:])
                part = EP.tile([128, BL], f32, tag="part")
                nc.vector.reduce_sum(
                    part[:],
                    masked.rearrange("p (j b) -> p b j", b=BL),
                    axis=AX.X,
                )
                accp = PEP.tile([BL, 1], f32, tag="accp")
                nc.tensor.matmul(accp[:], part[:], ones128[:], start=True, stop=True)
                lp_sb = EP.tile([BL, 1], f32, tag="lp_sb")
                nc.vector.tensor_copy(lp_sb[:], accp[:])
                nc.sync.dma_start(out=d_olp.ap(), in_=lp_sb[:])

                ksel = EP.tile([128, NT_ * BK], f32, tag="ksel")
                nc.vector.tensor_mul(ksel[:], oneb_f, kps[:])
                idxf = EP.tile([128, NT_ * BL], f32, tag="idxf")
                nc.vector.reduce_sum(
                    idxf[:],
                    ksel.rearrange("p (j b k) -> p j b k", b=BL, k=K6),
                    axis=AX.X,
                )
                idxi = EP.tile([128, NT_ * BL], i32, tag="idxi")
                nc.vector.tensor_copy(idxi[:], idxf[:])
                for j in range(NT_):
                    pmax = min(128, T_ - j * 128)
                    nc.sync.dma_start(
                        out=d_oidx.ap()[:, j * 128 : j * 128 + pmax].transpose(
                            [1, 0]
                        ),
                        in_=idxi[0:pmax, j * BL : (j + 1) * BL],
                    )

    nc.compile()
    return nc


def _prep_shared(inputs, T_):
    """Host-side weight prep (shared across cores)."""
    f = np.float32
    l1_w = inputs["l1_w"].astype(f)
    p1_w = inputs["p1_w"].astype(f)
    w_ih = inputs["w_ih"].astype(f)
    w_hh = inputs["w_hh"].astype(f)
    bc = (inputs["b_ih"] + inputs["b_hh"]).astype(f)
    NT_ = (T_ + 127) // 128
    gorder = [0, 1, 3, 2]  # i, f, o, g
    wcfT = np.empty((128, 4 * C), f)
    wcoT = np.empty((128, 4 * C), f)
    bcr = np.empty((4, C), f)
    for gi, g in enumerate(gorder):
        rows = slice(g * C, (g + 1) * C)
        s5 = f(0.5) if gi < 3 else f(1.0)  # sigmoid-as-tanh pre-scale (i,f,o)
        wcfT[:, gi * C : (gi + 1) * C] = s5 * w_ih[rows, :H].T
        wcoT[:, gi * C : (gi + 1) * C] = s5 * (w_ih[rows, H:] + w_hh[rows, :]).T
        bcr[gi, :] = s5 * bc[rows]
    kcol = np.tile(np.arange(K6, dtype=f), BL)  # [48]
    kp = np.broadcast_to(kcol, (128, NT_, BK)).reshape(128, NT_ * BK).copy()
    bcT = bcr.T.copy()
    return {
        "bcT": bcT,
        "l1wT": np.ascontiguousarray(l1_w.T),
        "l1b6": np.ascontiguousarray(inputs["l1_b"].astype(f).reshape(6, 128).T),
        "p1cfT": np.ascontiguousarray(p1_w[:, :H].T),
        "p1hT": np.ascontiguousarray(p1_w[:, H:].T),
        "p1b": inputs["p1_b"].astype(f).reshape(128, 1).copy(),
        "p2wT": np.ascontiguousarray(inputs["p2_w"].astype(f).T),
        "wcfT": wcfT,
        "wcoT": wcoT,
        "bcr": bcr,
        "e46": np.kron(np.eye(4, dtype=f), np.ones((1, BK), f)),
        "id128": np.eye(128, dtype=f),
        "kp": kp,
    }


def _rot(x_t, T_, width):
    """[T, width] -> rotated [128, NT*width] (partition = t%128, block = t//128)."""
    f = np.float32
    NT_ = (T_ + 127) // 128
    Xp = np.zeros((NT_ * 128, width), f)
    Xp[:T_] = x_t
    return np.ascontiguousarray(
        Xp.reshape(NT_, 128, width).transpose(1, 0, 2).reshape(128, NT_ * width)
    )


def _prep_core(inputs, ci, T_):
    f = np.float32
    p2b = f(np.asarray(inputs["p2_b"]).reshape(-1)[0])
    sl = slice(ci * BL, (ci + 1) * BL)
    mem = inputs["memory"][sl, 2 : 2 + T_, :].astype(f)  # [8, T, 256]
    memT = np.ascontiguousarray(mem.transpose(2, 0, 1).reshape(E, BL * T_))
    mask_t = inputs["mask"][sl, 1 : 1 + T_, :]  # [8, T, 6]
    SMt = np.where(mask_t, f(0.0), NEG).astype(f).transpose(1, 0, 2)  # [T, 8, 6]
    gum = inputs["gumbel"][:T_, sl, :].astype(f)  # [T, 8, 6]
    gp2 = (gum + SMt + p2b).reshape(1, T_ * BK).astype(f)
    gmr = _rot(gum.reshape(T_, BK), T_, BK)
    length = inputs["length"][sl].astype(np.int64)
    Mt = (length[None, :] > (np.arange(T_) + 1)[:, None]).astype(f)  # [T, 8]
    mb = _rot(Mt, T_, BL)
    return {"memT": memT, "gp2": gp2, "gmr": gmr, "mb": mb}


def _make_in_maps(inputs, T_):
    shared = _prep_shared(inputs, T_)
    return [dict(shared, **_prep_core(inputs, ci, T_)) for ci in range(NCORES)]


LAST_RESULTS = None


def kernel(**inputs):
    global LAST_RESULTS
    from concourse.bass_utils import run_bass_kernel_spmd

    inputs = {k: np.asarray(v) for k, v in inputs.items()}
    in_maps = _make_in_maps(inputs, T)
    nc = _build_program(float(np.asarray(inputs["p2_b"]).reshape(-1)[0]), T)
    res = run_bass_kernel_spmd(nc, in_maps, core_ids=list(range(NCORES)))
    LAST_RESULTS = res
    idx = np.concatenate([r["out_idx"] for r in res.results], axis=0)
    lp = np.concatenate([r["out_lp"][:, 0] for r in res.results], axis=0)
    return idx.astype(np.int32), lp.astype(np.float32)


# revision 47
# speedup vs baseline: 1.1650x; 1.0751x over previous
"""Trainium2 Bass kernel for the Actor sampling module (nn_Actor_47588237640247).

Strategy: pure data-parallel across 8 NeuronCores (8 samples each). Per core:
  Phase 1: choice features cf = relu(l1(memory)) computed once into SBUF in a
           transposed layout cfT[h, (t, b, k)] (fp32, ~12.6 MB).
  Scan:    511 fully-unrolled steps, run as TWO interleaved independent chains
           (samples 0-3 / 4-7) so the engines overlap the chains' serial
           latencies. Everything lives transposed (feature dim on partitions,
           (b,k) on the free axis); per step and half:
             hid = relu(P1cf @ cf_t + P1h @ h + b)          (PE x2 + ACT relu)
             raw = p2 @ hid                                  (PE -> [1,24])
             noisy = max(raw, -p2b) + (gumbel+maskneg+p2b)   (DVE stt, staged out)
             onehot = (noisy >= rowmax)                      (DVE reduce + cmp)
             chosen = sum_k onehot * cf_t                    (PE bcast + DVE x2)
             gates  = bias + Wcomb @ h + Wcf @ chosen        (PE, gates transposed)
             LSTM cell                                       (ACT sigmoid/tanh + DVE)
           Per-step rows (noisy scores, onehot) are staged in [1, 8*48] blocks
           and DMA'd every 8 steps into rotated [128, 4*48] buffers.
  End:     logits = noisy - gumbel; batched log-softmax gather (exp/ln on ACT)
           + masked sum -> log_probs; argmax decoded from onehot -> idx.
Host side only reshapes/transposes inputs and bakes gumbel+mask+bias tensors.
"""

import os
import sys

import numpy as np

for _p in ("/opt/trn_rl_repo", "/root/.axon_site/_ro/trn_rl_repo"):
    if os.path.isdir(_p) and _p not in sys.path:
        sys.path.append(_p)

B, L, E, H, C = 64, 512, 256, 128, 128
T = L - 1  # 511
NCORES = 8
BL = B // NCORES  # 8 samples per core
K6 = 6
BK = BL * K6  # 48
HB = BL // 2  # 4 samples per half-chain
HK = HB * K6  # 24
NEG = np.float32(-1.0e30)


def _build_program(p2b_val: float, T_: int):
    import concourse.bass as bass
    import concourse.tile as tile
    from concourse import bacc, mybir

    f32 = mybir.dt.float32
    i32 = mybir.dt.int32
    AL = mybir.AluOpType
    AF = mybir.ActivationFunctionType
    AX = mybir.AxisListType

    NT_ = (T_ + 127) // 128  # rotated-layout column blocks

    nc = bacc.Bacc(
        "TRN2", target_bir_lowering=False, debug=False, num_devices=NCORES
    )

    d_memT = nc.dram_tensor("memT", [E, BL * T_], f32, kind="ExternalInput")
    d_gp2 = nc.dram_tensor("gp2", [1, T_ * BK], f32, kind="ExternalInput")
    d_gmr = nc.dram_tensor("gmr", [128, NT_ * BK], f32, kind="ExternalInput")
    d_mb = nc.dram_tensor("mb", [128, NT_ * BL], f32, kind="ExternalInput")
    d_kp = nc.dram_tensor("kp", [128, NT_ * BK], f32, kind="ExternalInput")
    d_l1wT = nc.dram_tensor("l1wT", [E, 6 * H], f32, kind="ExternalInput")
    d_l1b6 = nc.dram_tensor("l1b6", [128, 6], f32, kind="ExternalInput")
    d_p1cfT = nc.dram_tensor("p1cfT", [128, 128], f32, kind="ExternalInput")
    d_p1hT = nc.dram_tensor("p1hT", [128, 128], f32, kind="ExternalInput")
    d_p1b = nc.dram_tensor("p1b", [128, 1], f32, kind="ExternalInput")
    d_p2wT = nc.dram_tensor("p2wT", [128, 1], f32, kind="ExternalInput")
    d_wcfT = nc.dram_tensor("wcfT", [128, 4 * C], f32, kind="ExternalInput")
    d_wcoT = nc.dram_tensor("wcoT", [128, 4 * C], f32, kind="ExternalInput")
    d_bcr = nc.dram_tensor("bcr", [4, C], f32, kind="ExternalInput")
    d_bcT = nc.dram_tensor("bcT", [C, 4], f32, kind="ExternalInput")
    d_e46 = nc.dram_tensor("e46", [4, 4 * BK], f32, kind="ExternalInput")
    d_id128 = nc.dram_tensor("id128", [128, 128], f32, kind="ExternalInput")
    d_oidx = nc.dram_tensor("out_idx", [BL, T_], i32, kind="ExternalOutput")
    d_olp = nc.dram_tensor("out_lp", [BL, 1], f32, kind="ExternalOutput")

    with tile.TileContext(nc) as tc:
        with (
            tc.tile_pool(name="persist", bufs=1) as P,
            tc.tile_pool(name="blk", bufs=3) as BP,
            tc.tile_pool(name="work", bufs=4) as W,
        ):
            # ---- persistent SBUF tiles
            l1b6 = P.tile([128, 6], f32, tag="l1b6")
            cft = P.tile([128, T_, BK], f32, tag="cft")
            p1cf = P.tile([128, 128], f32, tag="p1cf")
            p1h = P.tile([128, 128], f32, tag="p1h")
            p1b = P.tile([128, 1], f32, tag="p1b")
            p2w = P.tile([128, 1], f32, tag="p2w")
            wcf = P.tile([128, 4 * C], f32, tag="wcf")
            wco = P.tile([128, 4 * C], f32, tag="wco")
            bcr4 = P.tile([4, C], f32, tag="bcr4")
            bcT = P.tile([C, 4], f32, tag="bcT")
            e46 = P.tile([4, 4 * BK], f32, tag="e46")
            id128 = P.tile([128, 128], f32, tag="id128")
            gmr = P.tile([128, NT_ * BK], f32, tag="gmr")
            mbs = P.tile([128, NT_ * BL], f32, tag="mbs")
            kps = P.tile([128, NT_ * BK], f32, tag="kps")
            scb = P.tile([128, NT_, BK], f32, tag="scb")
            oneb = P.tile([128, NT_, BK], f32, tag="oneb")

            ones1128 = P.tile([1, 128], f32, tag="ones1128")
            ones128 = P.tile([128, 1], f32, tag="ones128")
            zero128 = P.tile([128, 1], f32, tag="zero128")

            # ---- input DMAs
            nc.sync.dma_start(out=l1b6[:], in_=d_l1b6.ap())
            nc.sync.dma_start(out=p1cf[:], in_=d_p1cfT.ap())
            nc.sync.dma_start(out=p1h[:], in_=d_p1hT.ap())
            nc.sync.dma_start(out=p1b[:], in_=d_p1b.ap())
            nc.sync.dma_start(out=p2w[:], in_=d_p2wT.ap())
            nc.sync.dma_start(out=wcf[:], in_=d_wcfT.ap())
            nc.sync.dma_start(out=wco[:], in_=d_wcoT.ap())
            nc.sync.dma_start(out=bcr4[:], in_=d_bcr.ap())
            nc.sync.dma_start(out=bcT[:], in_=d_bcT.ap())
            nc.sync.dma_start(out=e46[:], in_=d_e46.ap())
            nc.sync.dma_start(out=id128[:], in_=d_id128.ap())
            nc.sync.dma_start(out=gmr[:], in_=d_gmr.ap())
            nc.sync.dma_start(out=mbs[:], in_=d_mb.ap())
            nc.sync.dma_start(out=kps[:], in_=d_kp.ap())


            nc.vector.memset(ones1128[:], 1.0)
            nc.vector.memset(ones128[:], 1.0)
            nc.vector.memset(zero128[:], 0.0)
            nc.vector.memset(scb[:], 0.0)
            nc.vector.memset(oneb[:], 0.0)

            # ---- phase 1: cf = relu(l1 @ mem + b), scattered into cfT layout
            with (
                tc.tile_pool(name="ph1sbuf", bufs=1) as P1S,
                tc.tile_pool(name="ph1psum", bufs=4, space="PSUM") as PP1,
            ):
                memT0 = P1S.tile([128, BL * T_], f32, tag="memT0")
                memT1 = P1S.tile([128, BL * T_], f32, tag="memT1")
                l1w0 = P1S.tile([128, 6 * H], f32, tag="l1w0")
                l1w1 = P1S.tile([128, 6 * H], f32, tag="l1w1")
                nc.sync.dma_start(out=memT0[:], in_=d_memT.ap()[0:128, :])
                nc.sync.dma_start(out=memT1[:], in_=d_memT.ap()[128:256, :])
                nc.sync.dma_start(out=l1w0[:], in_=d_l1wT.ap()[0:128, :])
                nc.sync.dma_start(out=l1w1[:], in_=d_l1wT.ap()[128:256, :])
                for b in range(BL):
                    for k in range(K6):
                        pC = PP1.tile([128, T_], f32, tag="pC")
                        nc.tensor.matmul(
                            pC[:],
                            l1w0[:, k * H : (k + 1) * H],
                            memT0[:, b * T_ : (b + 1) * T_],
                            start=True,
                            stop=False,
                        )
                        nc.tensor.matmul(
                            pC[:],
                            l1w1[:, k * H : (k + 1) * H],
                            memT1[:, b * T_ : (b + 1) * T_],
                            start=False,
                            stop=True,
                        )
                        nc.vector.tensor_scalar(
                            cft[:, :, b * K6 + k],
                            pC[:],
                            l1b6[:, k : k + 1],
                            0.0,
                            AL.add,
                            AL.max,
                        )

            # ---- scan: full-width chain; speculative LSTM over all 6 candidates
            # off-chain: gates_k = bias + Wco@h + (Wcf@cf_k pre-batched); LSTM for
            # all k; chain only: hid-relu -> scores -> noisy -> argmax -> select.
            MB = 16  # steps per staging block
            with tc.tile_pool(name="spsum", bufs=2, space="PSUM") as SP:
                hsel = W.tile([128, 2 * BL], f32, tag="hcsel", name="hsel0")
                nc.vector.memset(hsel[:], 0.0)
                csel0 = W.tile([128, BL], f32, tag="csel0", name="csel0")
                nc.vector.memset(csel0[:], 0.0)
                csel = csel0
                nb8 = None
                one16 = None
                gpblk = None
                gcf16 = None
                for t in range(T_):
                    m16 = t % MB
                    if m16 == 0:
                        nb = min(MB, T_ - t)
                        gpblk = BP.tile([1, MB * BK], f32, tag="gpblk")
                        nc.sync.dma_start(
                            out=gpblk[:, : nb * BK],
                            in_=d_gp2.ap()[:, t * BK : (t + nb) * BK],
                        )
                        nb8 = BP.tile([1, MB * BK], f32, tag="nb8")
                        one16 = BP.tile([1, MB * BK], f32, tag="one16")
                        # pre-batch Wcf @ cf for the block: gcf16[t', (g,b,k)]
                        gcf16 = BP.tile([128, MB, 4 * BK], f32, tag="gcf16", bufs=2)
                        for g in range(4):
                            for c2 in range(4):
                                t0c = c2 * 4
                                ch = min(4, nb - t0c)
                                if ch <= 0:
                                    continue
                                psC = SP.tile(
                                    [128, 8 * BK], f32, tag="psC", bufs=2, name="psC"
                                )
                                nc.tensor.matmul(
                                    psC[:, : ch * BK],
                                    wcf[:, g * C : (g + 1) * C],
                                    cft[:, t + t0c : t + t0c + ch, :].rearrange(
                                        "p a k -> p (a k)"
                                    ),
                                    start=True,
                                    stop=True,
                                )
                                nc.scalar.activation(
                                    gcf16[
                                        :, t0c : t0c + ch, g * BK : (g + 1) * BK
                                    ],
                                    psC[:, : ch * BK].rearrange(
                                        "p (a k) -> p a k", k=BK
                                    ),
                                    AF.Identity,
                                    bias=bcT[:, g : g + 1],
                                    scale=1.0,
                                )

                    cft_t = cft[:, t, :]  # [128, 48]
                    hT48b = (
                        hsel[:, 0:BL].unsqueeze(2).to_broadcast([128, BL, K6])
                    )
                    hT8 = hsel[:, 0:BL]

                    # hid pre-activation (chain) + gate psums (off-chain)
                    pA = SP.tile([128, BK], f32, tag="pA", bufs=2)
                    nc.tensor.matmul(pA[:], p1cf[:], cft_t, start=True, stop=False)
                    nc.tensor.matmul(
                        pA.rearrange("p (b k) -> p b k", k=K6),
                        p1h[:],
                        hT48b,
                        start=False,
                        stop=True,
                    )

                    pG48 = SP.tile([128, 4 * BK], f32, tag="pG48", bufs=2)
                    nc.tensor.matmul(
                        pG48[:], id128[:], gcf16[:, m16, :], start=True, stop=False
                    )
                    hT8b = hT8.unsqueeze(2).to_broadcast([128, BL, K6])
                    for g in range(4):
                        nc.tensor.matmul(
                            pG48[:, g * BK : (g + 1) * BK].rearrange(
                                "p (b k) -> p b k", k=K6
                            ),
                            wco[:, g * C : (g + 1) * C],
                            hT8b,
                            start=False,
                            stop=(g == 3),
                        )

                    # speculative LSTM for all 6 candidates (off-chain)
                    # i/f/o weights pre-scaled 0.5 on host: sigmoid(x) =
                    # sigmoid-table with scale=2 on the halved gates.
                    sig = W.tile([128, 3 * BK], f32, tag="sig")
                    nc.scalar.activation(
                        sig[:], pG48[:, 0 : 3 * BK], AF.Sigmoid, bias=zero128[:], scale=2.0
                    )
                    th = W.tile([128, 4 * BK], f32, tag="th")
                    nc.scalar.activation(
                        th[:, 3 * BK : 4 * BK],
                        pG48[:, 3 * BK : 4 * BK],
                        AF.Tanh,
                        bias=zero128[:],
                        scale=1.0,
                    )
                    hc_cand = W.tile([128, 2 * BK], f32, tag="hc_cand")
                    t1 = W.tile([128, BK], f32, tag="t1")
                    nc.vector.tensor_mul(
                        t1.rearrange("p (b k) -> p b k", k=K6),
                        sig[:, BK : 2 * BK].rearrange("p (b k) -> p b k", k=K6),
                        csel.unsqueeze(2).to_broadcast([128, BL, K6]),
                    )
                    t2 = W.tile([128, BK], f32, tag="t2")
                    nc.vector.tensor_mul(t2[:], sig[:, 0:BK], th[:, 3 * BK : 4 * BK])
                    nc.vector.tensor_add(hc_cand[:, BK : 2 * BK], t1[:], t2[:])
                    tcs = W.tile([128, BK], f32, tag="tcs")
                    nc.scalar.activation(
                        tcs[:], hc_cand[:, BK : 2 * BK], AF.Tanh, bias=zero128[:], scale=1.0
                    )

                    # chain: hid -> scores -> noisy -> argmax -> select h,c
                    hid = W.tile([128, BK], f32, tag="hid")
                    nc.vector.tensor_scalar(
                        hid[:], pA[:], p1b[:], 0.0, AL.add, AL.max
                    )
                    pS = SP.tile([1, BK], f32, tag="pS", bufs=1)
                    nc.tensor.matmul(pS[:], p2w[:], hid[:], start=True, stop=True)
                    ms = slice(m16 * BK, (m16 + 1) * BK)
                    nc.vector.scalar_tensor_tensor(
                        nb8[:, ms], pS[:], -p2b_val, gpblk[:, ms], AL.max, AL.add
                    )
                    nv = nb8[:, ms].rearrange("p (b k) -> p b k", k=K6)
                    nmax = W.tile([1, BL], f32, tag="nmax")
                    nc.vector.reduce_max(nmax[:], nv, axis=AX.X)
                    one_v = one16[:, ms].rearrange("p (b k) -> p b k", k=K6)
                    nmax_b = nmax.unsqueeze(2).to_broadcast([1, BL, K6])
                    nc.vector.tensor_tensor(one_v, nv, nmax_b, op=AL.is_ge)
                    pO2 = SP.tile([128, 2 * BK], f32, tag="pO2", bufs=1)
                    nc.tensor.matmul(
                        pO2[:],
                        ones1128[:],
                        one16[:, ms]
                        .rearrange("p (a k) -> p a k", a=1)
                        .to_broadcast([1, 2, BK]),
                        start=True,
                        stop=True,
                    )
                    tmp96 = W.tile([128, 2 * BK], f32, tag="tmp96")
                    nc.vector.tensor_mul(
                        hc_cand[:, 0:BK], sig[:, 2 * BK : 3 * BK], pO2[:, 0:BK]
                    )
                    nc.vector.tensor_mul(
                        tmp96[:, BK : 2 * BK],
                        hc_cand[:, BK : 2 * BK],
                        pO2[:, BK : 2 * BK],
                    )
                    nc.vector.tensor_mul(tmp96[:, 0:BK], hc_cand[:, 0:BK], tcs[:])
                    hcsel = W.tile([128, 2 * BL], f32, tag="hcsel")
                    nc.vector.reduce_sum(
                        hcsel[:],
                        tmp96.rearrange("p (a b k) -> p a b k", a=2, k=K6),
                        axis=AX.X,
                    )
                    hsel = hcsel
                    csel = hcsel[:, BL : 2 * BL]

                    if m16 == MB - 1 or t == T_ - 1:
                        nb = m16 + 1
                        p0 = (t - m16) % 128
                        jj = (t - m16) // 128
                        nc.sync.dma_start(
                            out=scb[p0 : p0 + nb, jj, :],
                            in_=nb8[:, : nb * BK].rearrange(
                                "p (n k) -> p n k", k=BK
                            ),
                        )
                        nc.sync.dma_start(
                            out=oneb[p0 : p0 + nb, jj, :],
                            in_=one16[:, : nb * BK].rearrange(
                                "p (n k) -> p n k", k=BK
                            ),
                        )

            # ---- end phase: log-probs and indices
            with (
                tc.tile_pool(name="endp", bufs=1) as EP,
                tc.tile_pool(name="endpsum", bufs=1, space="PSUM") as PEP,
            ):
                scb_f = scb.rearrange("p j k -> p (j k)")
                oneb_f = oneb.rearrange("p j k -> p (j k)")
                logits = EP.tile([128, NT_ * BK], f32, tag="logits")
                nc.vector.tensor_sub(logits[:], scb_f, gmr[:])
                expv = EP.tile([128, NT_ * BK], f32, tag="expv")
                nc.scalar.activation(
                    expv[:], logits[:], AF.Exp, bias=zero128[:], scale=1.0
                )
                esum = EP.tile([128, NT_ * BL], f32, tag="esum")
                nc.vector.reduce_sum(
                    esum[:],
                    expv.rearrange("p (j b k) -> p j b k", b=BL, k=K6),
                    axis=AX.X,
                )
                lse = EP.tile([128, NT_ * BL], f32, tag="lse")
                nc.scalar.activation(
                    lse[:], esum[:], AF.Ln, bias=zero128[:], scale=1.0
                )
                selp = EP.tile([128, NT_ * BK], f32, tag="selp")
                nc.vector.tensor_mul(selp[:], logits[:], oneb_f)
                sel = EP.tile([128, NT_ * BL], f32, tag="sel")
                nc.vector.reduce_sum(
                    sel[:],
                    selp.rearrange("p (j b k) -> p j b k", b=BL, k=K6),
                    axis=AX.X,
                )
                diff = EP.tile([128, NT_ * BL], f32, tag="diff")
                nc.vector.tensor_sub(diff[:], sel[:], lse[:])
                masked = EP.tile([128, NT_ * BL], f32, tag="masked")
                nc.vector.tensor_mul(masked[:], diff[:], mbs[:])
                part = EP.tile([128, BL], f32, tag="part")
                nc.vector.reduce_sum(
                    part[:],
                    masked.rearrange("p (j b) -> p b j", b=BL),
                    axis=AX.X,
                )
                accp = PEP.tile([BL, 1], f32, tag="accp")
                nc.tensor.matmul(accp[:], part[:], ones128[:], start=True, stop=True)
                lp_sb = EP.tile([BL, 1], f32, tag="lp_sb")
                nc.vector.tensor_copy(lp_sb[:], accp[:])
                nc.sync.dma_start(out=d_olp.ap(), in_=lp_sb[:])

                ksel = EP.tile([128, NT_ * BK], f32, tag="ksel")
                nc.vector.tensor_mul(ksel[:], oneb_f, kps[:])
                idxf = EP.tile([128, NT_ * BL], f32, tag="idxf")
                nc.vector.reduce_sum(
                    idxf[:],
                    ksel.rearrange("p (j b k) -> p j b k", b=BL, k=K6),
                    axis=AX.X,
                )
                idxi = EP.tile([128, NT_ * BL], i32, tag="idxi")
                nc.vector.tensor_copy(idxi[:], idxf[:])
                for j in range(NT_):
                    pmax = min(128, T_ - j * 128)
                    nc.sync.dma_start(
                        out=d_oidx.ap()[:, j * 128 : j * 128 + pmax].transpose(
                            [1, 0]
                        ),
                        in_=idxi[0:pmax, j * BL : (j + 1) * BL],
                    )

    nc.compile()
    return nc


def _prep_shared(inputs, T_):
    """Host-side weight prep (shared across cores)."""
    f = np.float32
    l1_w = inputs["l1_w"].astype(f)
    p1_w = inputs["p1_w"].astype(f)
    w_ih = inputs["w_ih"].astype(f)
    w_hh = inputs["w_hh"].astype(f)
    bc = (inputs["b_ih"] + inputs["b_hh"]).astype(f)
    NT_ = (T_ + 127) // 128
    gorder = [0, 1, 3, 2]  # i, f, o, g
    wcfT = np.empty((128, 4 * C), f)
    wcoT = np.empty((128, 4 * C), f)
    bcr = np.empty((4, C), f)
    for gi, g in enumerate(gorder):
        rows = slice(g * C, (g + 1) * C)
        s5 = f(0.5) if gi < 3 else f(1.0)  # sigmoid-as-tanh pre-scale (i,f,o)
        wcfT[:, gi * C : (gi + 1) * C] = s5 * w_ih[rows, :H].T
        wcoT[:, gi * C : (gi + 1) * C] = s5 * (w_ih[rows, H:] + w_hh[rows, :]).T
        bcr[gi, :] = s5 * bc[rows]
    kcol = np.tile(np.arange(K6, dtype=f), BL)  # [48]
    kp = np.broadcast_to(kcol, (128, NT_, BK)).reshape(128, NT_ * BK).copy()
    bcT = bcr.T.copy()
    return {
        "bcT": bcT,
        "l1wT": np.ascontiguousarray(l1_w.T),
        "l1b6": np.ascontiguousarray(inputs["l1_b"].astype(f).reshape(6, 128).T),
        "p1cfT": np.ascontiguousarray(p1_w[:, :H].T),
        "p1hT": np.ascontiguousarray(p1_w[:, H:].T),
        "p1b": inputs["p1_b"].astype(f).reshape(128, 1).copy(),
        "p2wT": np.ascontiguousarray(inputs["p2_w"].astype(f).T),
        "wcfT": wcfT,
        "wcoT": wcoT,
        "bcr": bcr,
        "e46": np.kron(np.eye(4, dtype=f), np.ones((1, BK), f)),
        "id128": np.eye(128, dtype=f),
        "kp": kp,
    }


def _rot(x_t, T_, width):
    """[T, width] -> rotated [128, NT*width] (partition = t%128, block = t//128)."""
    f = np.float32
    NT_ = (T_ + 127) // 128
    Xp = np.zeros((NT_ * 128, width), f)
    Xp[:T_] = x_t
    return np.ascontiguousarray(
        Xp.reshape(NT_, 128, width).transpose(1, 0, 2).reshape(128, NT_ * width)
    )


def _prep_core(inputs, ci, T_):
    f = np.float32
    p2b = f(np.asarray(inputs["p2_b"]).reshape(-1)[0])
    sl = slice(ci * BL, (ci + 1) * BL)
    mem = inputs["memory"][sl, 2 : 2 + T_, :].astype(f)  # [8, T, 256]
    memT = np.ascontiguousarray(mem.transpose(2, 0, 1).reshape(E, BL * T_))
    mask_t = inputs["mask"][sl, 1 : 1 + T_, :]  # [8, T, 6]
    SMt = np.where(mask_t, f(0.0), NEG).astype(f).transpose(1, 0, 2)  # [T, 8, 6]
    gum = inputs["gumbel"][:T_, sl, :].astype(f)  # [T, 8, 6]
    gp2 = (gum + SMt + p2b).reshape(1, T_ * BK).astype(f)
    gmr = _rot(gum.reshape(T_, BK), T_, BK)
    length = inputs["length"][sl].astype(np.int64)
    Mt = (length[None, :] > (np.arange(T_) + 1)[:, None]).astype(f)  # [T, 8]
    mb = _rot(Mt, T_, BL)
    return {"memT": memT, "gp2": gp2, "gmr": gmr, "mb": mb}


def _make_in_maps(inputs, T_):
    shared = _prep_shared(inputs, T_)
    return [dict(shared, **_prep_core(inputs, ci, T_)) for ci in range(NCORES)]


LAST_RESULTS = None


def kernel(**inputs):
    global LAST_RESULTS
    from concourse.bass_utils import run_bass_kernel_spmd

    inputs = {k: np.asarray(v) for k, v in inputs.items()}
    in_maps = _make_in_maps(inputs, T)
    nc = _build_program(float(np.asarray(inputs["p2_b"]).reshape(-1)[0]), T)
    res = run_bass_kernel_spmd(nc, in_maps, core_ids=list(range(NCORES)))
    LAST_RESULTS = res
    idx = np.concatenate([r["out_idx"] for r in res.results], axis=0)
    lp = np.concatenate([r["out_lp"][:, 0] for r in res.results], axis=0)
    return idx.astype(np.int32), lp.astype(np.float32)


# revision 48
# speedup vs baseline: 1.2365x; 1.0614x over previous
"""Trainium2 Bass kernel for the Actor sampling module (nn_Actor_47588237640247).

Strategy: pure data-parallel across 8 NeuronCores (8 samples each). Per core:
  Phase 1: choice features cf = relu(l1(memory)) computed once into SBUF in a
           transposed layout cfT[h, (t, b, k)] (fp32, ~12.6 MB).
  Scan:    511 fully-unrolled steps, run as TWO interleaved independent chains
           (samples 0-3 / 4-7) so the engines overlap the chains' serial
           latencies. Everything lives transposed (feature dim on partitions,
           (b,k) on the free axis); per step and half:
             hid = relu(P1cf @ cf_t + P1h @ h + b)          (PE x2 + ACT relu)
             raw = p2 @ hid                                  (PE -> [1,24])
             noisy = max(raw, -p2b) + (gumbel+maskneg+p2b)   (DVE stt, staged out)
             onehot = (noisy >= rowmax)                      (DVE reduce + cmp)
             chosen = sum_k onehot * cf_t                    (PE bcast + DVE x2)
             gates  = bias + Wcomb @ h + Wcf @ chosen        (PE, gates transposed)
             LSTM cell                                       (ACT sigmoid/tanh + DVE)
           Per-step rows (noisy scores, onehot) are staged in [1, 8*48] blocks
           and DMA'd every 8 steps into rotated [128, 4*48] buffers.
  End:     logits = noisy - gumbel; batched log-softmax gather (exp/ln on ACT)
           + masked sum -> log_probs; argmax decoded from onehot -> idx.
Host side only reshapes/transposes inputs and bakes gumbel+mask+bias tensors.
"""

import os
import sys

import numpy as np

for _p in ("/opt/trn_rl_repo", "/root/.axon_site/_ro/trn_rl_repo"):
    if os.path.isdir(_p) and _p not in sys.path:
        sys.path.append(_p)

B, L, E, H, C = 64, 512, 256, 128, 128
T = L - 1  # 511
NCORES = 8
BL = B // NCORES  # 8 samples per core
K6 = 6
BK = BL * K6  # 48
HB = BL // 2  # 4 samples per half-chain
HK = HB * K6  # 24
NEG = np.float32(-1.0e30)


def _build_program(p2b_val: float, T_: int):
    import concourse.bass as bass
    import concourse.tile as tile
    from concourse import bacc, mybir

    f32 = mybir.dt.float32
    i32 = mybir.dt.int32
    AL = mybir.AluOpType
    AF = mybir.ActivationFunctionType
    AX = mybir.AxisListType

    NT_ = (T_ + 127) // 128  # rotated-layout column blocks

    nc = bacc.Bacc(
        "TRN2", target_bir_lowering=False, debug=False, num_devices=NCORES
    )

    d_memT = nc.dram_tensor("memT", [E, BL * T_], f32, kind="ExternalInput")
    d_gp2 = nc.dram_tensor("gp2", [1, T_ * BK], f32, kind="ExternalInput")
    d_gmr = nc.dram_tensor("gmr", [128, NT_ * BK], f32, kind="ExternalInput")
    d_mb = nc.dram_tensor("mb", [128, NT_ * BL], f32, kind="ExternalInput")
    d_kp = nc.dram_tensor("kp", [128, NT_ * BK], f32, kind="ExternalInput")
    d_l1wT = nc.dram_tensor("l1wT", [E, 6 * H], f32, kind="ExternalInput")
    d_l1b6 = nc.dram_tensor("l1b6", [128, 6], f32, kind="ExternalInput")
    d_p1cfT = nc.dram_tensor("p1cfT", [128, 128], f32, kind="ExternalInput")
    d_p1hT = nc.dram_tensor("p1hT", [128, 128], f32, kind="ExternalInput")
    d_p1b = nc.dram_tensor("p1b", [128, 1], f32, kind="ExternalInput")
    d_p2wT = nc.dram_tensor("p2wT", [128, 1], f32, kind="ExternalInput")
    d_wcfT = nc.dram_tensor("wcfT", [128, 4 * C], f32, kind="ExternalInput")
    d_wcoT = nc.dram_tensor("wcoT", [128, 4 * C], f32, kind="ExternalInput")
    d_bcr = nc.dram_tensor("bcr", [4, C], f32, kind="ExternalInput")
    d_bcT = nc.dram_tensor("bcT", [C, 4], f32, kind="ExternalInput")
    d_e46 = nc.dram_tensor("e46", [4, 4 * BK], f32, kind="ExternalInput")
    d_id128 = nc.dram_tensor("id128", [128, 128], f32, kind="ExternalInput")
    d_oidx = nc.dram_tensor("out_idx", [BL, T_], i32, kind="ExternalOutput")
    d_olp = nc.dram_tensor("out_lp", [BL, 1], f32, kind="ExternalOutput")

    with tile.TileContext(nc) as tc:
        with (
            tc.tile_pool(name="persist", bufs=1) as P,
            tc.tile_pool(name="blk", bufs=3) as BP,
            tc.tile_pool(name="work", bufs=4) as W,
        ):
            # ---- persistent SBUF tiles
            l1b6 = P.tile([128, 6], f32, tag="l1b6")
            cft = P.tile([128, T_, BK], f32, tag="cft")
            p1cf = P.tile([128, 128], f32, tag="p1cf")
            p1h = P.tile([128, 128], f32, tag="p1h")
            p1b = P.tile([128, 1], f32, tag="p1b")
            p2w = P.tile([128, 1], f32, tag="p2w")
            wcf = P.tile([128, 4 * C], f32, tag="wcf")
            wco = P.tile([128, 4 * C], f32, tag="wco")
            bcr4 = P.tile([4, C], f32, tag="bcr4")
            bcT = P.tile([C, 4], f32, tag="bcT")
            e46 = P.tile([4, 4 * BK], f32, tag="e46")
            id128 = P.tile([128, 128], f32, tag="id128")
            gmr = P.tile([128, NT_ * BK], f32, tag="gmr")
            mbs = P.tile([128, NT_ * BL], f32, tag="mbs")
            kps = P.tile([128, NT_ * BK], f32, tag="kps")
            scb = P.tile([128, NT_, BK], f32, tag="scb")
            oneb = P.tile([128, NT_, BK], f32, tag="oneb")

            ones1128 = P.tile([1, 128], f32, tag="ones1128")
            ones128 = P.tile([128, 1], f32, tag="ones128")
            zero128 = P.tile([128, 1], f32, tag="zero128")

            # ---- input DMAs
            nc.sync.dma_start(out=l1b6[:], in_=d_l1b6.ap())
            nc.sync.dma_start(out=p1cf[:], in_=d_p1cfT.ap())
            nc.sync.dma_start(out=p1h[:], in_=d_p1hT.ap())
            nc.sync.dma_start(out=p1b[:], in_=d_p1b.ap())
            nc.sync.dma_start(out=p2w[:], in_=d_p2wT.ap())
            nc.sync.dma_start(out=wcf[:], in_=d_wcfT.ap())
            nc.sync.dma_start(out=wco[:], in_=d_wcoT.ap())
            nc.sync.dma_start(out=bcr4[:], in_=d_bcr.ap())
            nc.sync.dma_start(out=bcT[:], in_=d_bcT.ap())
            nc.sync.dma_start(out=e46[:], in_=d_e46.ap())
            nc.sync.dma_start(out=id128[:], in_=d_id128.ap())
            nc.sync.dma_start(out=gmr[:], in_=d_gmr.ap())
            nc.sync.dma_start(out=mbs[:], in_=d_mb.ap())
            nc.sync.dma_start(out=kps[:], in_=d_kp.ap())


            nc.vector.memset(ones1128[:], 1.0)
            nc.vector.memset(ones128[:], 1.0)
            nc.vector.memset(zero128[:], 0.0)
            nc.vector.memset(scb[:], 0.0)
            nc.vector.memset(oneb[:], 0.0)

            # ---- phase 1: cf = relu(l1 @ mem + b), scattered into cfT layout
            with (
                tc.tile_pool(name="ph1sbuf", bufs=1) as P1S,
                tc.tile_pool(name="ph1psum", bufs=4, space="PSUM") as PP1,
            ):
                memT0 = P1S.tile([128, BL * T_], f32, tag="memT0")
                memT1 = P1S.tile([128, BL * T_], f32, tag="memT1")
                l1w0 = P1S.tile([128, 6 * H], f32, tag="l1w0")
                l1w1 = P1S.tile([128, 6 * H], f32, tag="l1w1")
                nc.sync.dma_start(out=memT0[:], in_=d_memT.ap()[0:128, :])
                nc.sync.dma_start(out=memT1[:], in_=d_memT.ap()[128:256, :])
                nc.sync.dma_start(out=l1w0[:], in_=d_l1wT.ap()[0:128, :])
                nc.sync.dma_start(out=l1w1[:], in_=d_l1wT.ap()[128:256, :])
                for b in range(BL):
                    for k in range(K6):
                        pC = PP1.tile([128, T_], f32, tag="pC")
                        nc.tensor.matmul(
                            pC[:],
                            l1w0[:, k * H : (k + 1) * H],
                            memT0[:, b * T_ : (b + 1) * T_],
                            start=True,
                            stop=False,
                        )
                        nc.tensor.matmul(
                            pC[:],
                            l1w1[:, k * H : (k + 1) * H],
                            memT1[:, b * T_ : (b + 1) * T_],
                            start=False,
                            stop=True,
                        )
                        nc.vector.tensor_scalar(
                            cft[:, :, b * K6 + k],
                            pC[:],
                            l1b6[:, k : k + 1],
                            0.0,
                            AL.add,
                            AL.max,
                        )

            # ---- scan: full-width chain; speculative LSTM over all 6 candidates
            # off-chain: gates_k = bias + Wco@h + (Wcf@cf_k pre-batched); LSTM for
            # all k; chain only: hid-relu -> scores -> noisy -> argmax -> select.
            MB = 16  # steps per staging block
            with tc.tile_pool(name="spsum", bufs=2, space="PSUM") as SP:
                hsel = W.tile([128, 2 * BL], f32, tag="hcsel", name="hsel0")
                nc.vector.memset(hsel[:], 0.0)
                csel0 = W.tile([128, BL], f32, tag="csel0", name="csel0")
                nc.vector.memset(csel0[:], 0.0)
                csel = csel0
                nb8 = None
                one16 = None
                gpblk = None
                gcf16 = None
                for t in range(T_):
                    m16 = t % MB
                    if m16 == 0:
                        nb = min(MB, T_ - t)
                        gpblk = BP.tile([1, MB * BK], f32, tag="gpblk")
                        nc.sync.dma_start(
                            out=gpblk[:, : nb * BK],
                            in_=d_gp2.ap()[:, t * BK : (t + nb) * BK],
                        )
                        nb8 = BP.tile([1, MB * BK], f32, tag="nb8")
                        one16 = BP.tile([1, MB * BK], f32, tag="one16")
                        # pre-batch Wcf @ cf for the block: gcf16[t', (g,b,k)]
                        gcf16 = BP.tile([128, MB, 4 * BK], f32, tag="gcf16", bufs=2)
                        for g in range(4):
                            for c2 in range(4):
                                t0c = c2 * 4
                                ch = min(4, nb - t0c)
                                if ch <= 0:
                                    continue
                                psC = SP.tile(
                                    [128, 8 * BK], f32, tag="psC", bufs=2, name="psC"
                                )
                                nc.tensor.matmul(
                                    psC[:, : ch * BK],
                                    wcf[:, g * C : (g + 1) * C],
                                    cft[:, t + t0c : t + t0c + ch, :].rearrange(
                                        "p a k -> p (a k)"
                                    ),
                                    start=True,
                                    stop=True,
                                )
                                nc.scalar.activation(
                                    gcf16[
                                        :, t0c : t0c + ch, g * BK : (g + 1) * BK
                                    ],
                                    psC[:, : ch * BK].rearrange(
                                        "p (a k) -> p a k", k=BK
                                    ),
                                    AF.Identity,
                                    bias=bcT[:, g : g + 1],
                                    scale=1.0,
                                )

                    cft_t = cft[:, t, :]  # [128, 48]
                    hT48b = (
                        hsel[:, 0:BL].unsqueeze(2).to_broadcast([128, BL, K6])
                    )
                    hT8 = hsel[:, 0:BL]

                    # hid pre-activation (chain) + gate psums (off-chain)
                    pA = SP.tile([128, BK], f32, tag="pA", bufs=2)
                    nc.tensor.matmul(pA[:], p1cf[:], cft_t, start=True, stop=False)
                    nc.tensor.matmul(
                        pA.rearrange("p (b k) -> p b k", k=K6),
                        p1h[:],
                        hT48b,
                        start=False,
                        stop=True,
                    )

                    pGifo = SP.tile([128, 3 * BK], f32, tag="pGifo", bufs=1)
                    pGg = SP.tile([128, BK], f32, tag="pGg", bufs=1)
                    nc.tensor.matmul(
                        pGifo[:],
                        id128[:],
                        gcf16[:, m16, 0 : 3 * BK],
                        start=True,
                        stop=False,
                    )
                    nc.tensor.matmul(
                        pGg[:],
                        id128[:],
                        gcf16[:, m16, 3 * BK : 4 * BK],
                        start=True,
                        stop=False,
                    )
                    hT8b = hT8.unsqueeze(2).to_broadcast([128, BL, K6])
                    for g in range(3):
                        nc.tensor.matmul(
                            pGifo[:, g * BK : (g + 1) * BK].rearrange(
                                "p (b k) -> p b k", k=K6
                            ),
                            wco[:, g * C : (g + 1) * C],
                            hT8b,
                            start=False,
                            stop=(g == 2),
                        )
                    nc.tensor.matmul(
                        pGg.rearrange("p (b k) -> p b k", k=K6),
                        wco[:, 3 * C : 4 * C],
                        hT8b,
                        start=False,
                        stop=True,
                    )

                    # speculative LSTM for all 6 candidates (off-chain)
                    # i/f/o weights pre-scaled 0.5 on host: sigmoid(x) =
                    # sigmoid-table with scale=2 on the halved gates.
                    sig = W.tile([128, 3 * BK], f32, tag="sig")
                    nc.scalar.activation(
                        sig[:], pGifo[:], AF.Sigmoid, bias=zero128[:], scale=2.0
                    )
                    th = W.tile([128, 4 * BK], f32, tag="th")
                    nc.scalar.activation(
                        th[:, 3 * BK : 4 * BK],
                        pGg[:],
                        AF.Tanh,
                        bias=zero128[:],
                        scale=1.0,
                    )
                    hc_cand = W.tile([128, 2 * BK], f32, tag="hc_cand")
                    t1 = W.tile([128, BK], f32, tag="t1")
                    nc.vector.tensor_mul(
                        t1.rearrange("p (b k) -> p b k", k=K6),
                        sig[:, BK : 2 * BK].rearrange("p (b k) -> p b k", k=K6),
                        csel.unsqueeze(2).to_broadcast([128, BL, K6]),
                    )
                    t2 = W.tile([128, BK], f32, tag="t2")
                    nc.vector.tensor_mul(t2[:], sig[:, 0:BK], th[:, 3 * BK : 4 * BK])
                    nc.vector.tensor_add(hc_cand[:, BK : 2 * BK], t1[:], t2[:])
                    tcs = W.tile([128, BK], f32, tag="tcs")
                    nc.scalar.activation(
                        tcs[:], hc_cand[:, BK : 2 * BK], AF.Tanh, bias=zero128[:], scale=1.0
                    )

                    # chain: hid -> scores -> noisy -> argmax -> select h,c
                    hid = W.tile([128, BK], f32, tag="hid")
                    nc.vector.tensor_scalar(
                        hid[:], pA[:], p1b[:], 0.0, AL.add, AL.max
                    )
                    pS = SP.tile([1, BK], f32, tag="pS", bufs=1)
                    nc.tensor.matmul(pS[:], p2w[:], hid[:], start=True, stop=True)
                    ms = slice(m16 * BK, (m16 + 1) * BK)
                    nc.vector.scalar_tensor_tensor(
                        nb8[:, ms], pS[:], -p2b_val, gpblk[:, ms], AL.max, AL.add
                    )
                    nv = nb8[:, ms].rearrange("p (b k) -> p b k", k=K6)
                    nmax = W.tile([1, BL], f32, tag="nmax")
                    nc.vector.reduce_max(nmax[:], nv, axis=AX.X)
                    one_v = one16[:, ms].rearrange("p (b k) -> p b k", k=K6)
                    nmax_b = nmax.unsqueeze(2).to_broadcast([1, BL, K6])
                    nc.vector.tensor_tensor(one_v, nv, nmax_b, op=AL.is_ge)
                    pO2 = SP.tile([128, 2 * BK], f32, tag="pO2", bufs=1)
                    nc.tensor.matmul(
                        pO2[:],
                        ones1128[:],
                        one16[:, ms]
                        .rearrange("p (a k) -> p a k", a=1)
                        .to_broadcast([1, 2, BK]),
                        start=True,
                        stop=True,
                    )
                    tmp96 = W.tile([128, 2 * BK], f32, tag="tmp96")
                    nc.vector.tensor_mul(
                        hc_cand[:, 0:BK], sig[:, 2 * BK : 3 * BK], pO2[:, 0:BK]
                    )
                    nc.vector.tensor_mul(
                        tmp96[:, BK : 2 * BK],
                        hc_cand[:, BK : 2 * BK],
                        pO2[:, BK : 2 * BK],
                    )
                    nc.vector.tensor_mul(tmp96[:, 0:BK], hc_cand[:, 0:BK], tcs[:])
                    hcsel = W.tile([128, 2 * BL], f32, tag="hcsel")
                    nc.vector.reduce_sum(
                        hcsel[:],
                        tmp96.rearrange("p (a b k) -> p a b k", a=2, k=K6),
                        axis=AX.X,
                    )
                    hsel = hcsel
                    csel = hcsel[:, BL : 2 * BL]

                    if m16 == MB - 1 or t == T_ - 1:
                        nb = m16 + 1
                        p0 = (t - m16) % 128
                        jj = (t - m16) // 128
                        nc.sync.dma_start(
                            out=scb[p0 : p0 + nb, jj, :],
                            in_=nb8[:, : nb * BK].rearrange(
                                "p (n k) -> p n k", k=BK
                            ),
                        )
                        nc.sync.dma_start(
                            out=oneb[p0 : p0 + nb, jj, :],
                            in_=one16[:, : nb * BK].rearrange(
                                "p (n k) -> p n k", k=BK
                            ),
                        )

            # ---- end phase: log-probs and indices
            with (
                tc.tile_pool(name="endp", bufs=1) as EP,
                tc.tile_pool(name="endpsum", bufs=1, space="PSUM") as PEP,
            ):
                scb_f = scb.rearrange("p j k -> p (j k)")
                oneb_f = oneb.rearrange("p j k -> p (j k)")
                logits = EP.tile([128, NT_ * BK], f32, tag="logits")
                nc.vector.tensor_sub(logits[:], scb_f, gmr[:])
                expv = EP.tile([128, NT_ * BK], f32, tag="expv")
                nc.scalar.activation(
                    expv[:], logits[:], AF.Exp, bias=zero128[:], scale=1.0
                )
                esum = EP.tile([128, NT_ * BL], f32, tag="esum")
                nc.vector.reduce_sum(
                    esum[:],
                    expv.rearrange("p (j b k) -> p j b k", b=BL, k=K6),
                    axis=AX.X,
                )
                lse = EP.tile([128, NT_ * BL], f32, tag="lse")
                nc.scalar.activation(
                    lse[:], esum[:], AF.Ln, bias=zero128[:], scale=1.0
                )
                selp = EP.tile([128, NT_ * BK], f32, tag="selp")
                nc.vector.tensor_mul(selp[:], logits[:], oneb_f)
                sel = EP.tile([128, NT_ * BL], f32, tag="sel")
                nc.vector.reduce_sum(
                    sel[:],
                    selp.rearrange("p (j b k) -> p j b k", b=BL, k=K6),
                    axis=AX.X,
                )
                diff = EP.tile([128, NT_ * BL], f32, tag="diff")
                nc.vector.tensor_sub(diff[:], sel[:], lse[:])
                masked = EP.tile([128, NT_ * BL], f32, tag="masked")
                nc.vector.tensor_mul(masked[:], diff[:], mbs[:])
                part = EP.tile([128, BL], f32, tag="part")
                nc.vector.reduce_sum(
                    part[:],
                    masked.rearrange("p (j b) -> p b j", b=BL),
                    axis=AX.X,
                )
                accp = PEP.tile([BL, 1], f32, tag="accp")
                nc.tensor.matmul(accp[:], part[:], ones128[:], start=True, stop=True)
                lp_sb = EP.tile([BL, 1], f32, tag="lp_sb")
                nc.vector.tensor_copy(lp_sb[:], accp[:])
                nc.sync.dma_start(out=d_olp.ap(), in_=lp_sb[:])

                ksel = EP.tile([128, NT_ * BK], f32, tag="ksel")
                nc.vector.tensor_mul(ksel[:], oneb_f, kps[:])
                idxf = EP.tile([128, NT_ * BL], f32, tag="idxf")
                nc.vector.reduce_sum(
                    idxf[:],
                    ksel.rearrange("p (j b k) -> p j b k", b=BL, k=K6),
                    axis=AX.X,
                )
                idxi = EP.tile([128, NT_ * BL], i32, tag="idxi")
                nc.vector.tensor_copy(idxi[:], idxf[:])
                for j in range(NT_):
                    pmax = min(128, T_ - j * 128)
                    nc.sync.dma_start(
                        out=d_oidx.ap()[:, j * 128 : j * 128 + pmax].transpose(
                            [1, 0]
                        ),
                        in_=idxi[0:pmax, j * BL : (j + 1) * BL],
                    )

    nc.compile()
    return nc


def _prep_shared(inputs, T_):
    """Host-side weight prep (shared across cores)."""
    f = np.float32
    l1_w = inputs["l1_w"].astype(f)
    p1_w = inputs["p1_w"].astype(f)
    w_ih = inputs["w_ih"].astype(f)
    w_hh = inputs["w_hh"].astype(f)
    bc = (inputs["b_ih"] + inputs["b_hh"]).astype(f)
    NT_ = (T_ + 127) // 128
    gorder = [0, 1, 3, 2]  # i, f, o, g
    wcfT = np.empty((128, 4 * C), f)
    wcoT = np.empty((128, 4 * C), f)
    bcr = np.empty((4, C), f)
    for gi, g in enumerate(gorder):
        rows = slice(g * C, (g + 1) * C)
        s5 = f(0.5) if gi < 3 else f(1.0)  # sigmoid-as-tanh pre-scale (i,f,o)
        wcfT[:, gi * C : (gi + 1) * C] = s5 * w_ih[rows, :H].T
        wcoT[:, gi * C : (gi + 1) * C] = s5 * (w_ih[rows, H:] + w_hh[rows, :]).T
        bcr[gi, :] = s5 * bc[rows]
    kcol = np.tile(np.arange(K6, dtype=f), BL)  # [48]
    kp = np.broadcast_to(kcol, (128, NT_, BK)).reshape(128, NT_ * BK).copy()
    bcT = bcr.T.copy()
    return {
        "bcT": bcT,
        "l1wT": np.ascontiguousarray(l1_w.T),
        "l1b6": np.ascontiguousarray(inputs["l1_b"].astype(f).reshape(6, 128).T),
        "p1cfT": np.ascontiguousarray(p1_w[:, :H].T),
        "p1hT": np.ascontiguousarray(p1_w[:, H:].T),
        "p1b": inputs["p1_b"].astype(f).reshape(128, 1).copy(),
        "p2wT": np.ascontiguousarray(inputs["p2_w"].astype(f).T),
        "wcfT": wcfT,
        "wcoT": wcoT,
        "bcr": bcr,
        "e46": np.kron(np.eye(4, dtype=f), np.ones((1, BK), f)),
        "id128": np.eye(128, dtype=f),
        "kp": kp,
    }


def _rot(x_t, T_, width):
    """[T, width] -> rotated [128, NT*width] (partition = t%128, block = t//128)."""
    f = np.float32
    NT_ = (T_ + 127) // 128
    Xp = np.zeros((NT_ * 128, width), f)
    Xp[:T_] = x_t
    return np.ascontiguousarray(
        Xp.reshape(NT_, 128, width).transpose(1, 0, 2).reshape(128, NT_ * width)
    )


def _prep_core(inputs, ci, T_):
    f = np.float32
    p2b = f(np.asarray(inputs["p2_b"]).reshape(-1)[0])
    sl = slice(ci * BL, (ci + 1) * BL)
    mem = inputs["memory"][sl, 2 : 2 + T_, :].astype(f)  # [8, T, 256]
    memT = np.ascontiguousarray(mem.transpose(2, 0, 1).reshape(E, BL * T_))
    mask_t = inputs["mask"][sl, 1 : 1 + T_, :]  # [8, T, 6]
    SMt = np.where(mask_t, f(0.0), NEG).astype(f).transpose(1, 0, 2)  # [T, 8, 6]
    gum = inputs["gumbel"][:T_, sl, :].astype(f)  # [T, 8, 6]
    gp2 = (gum + SMt + p2b).reshape(1, T_ * BK).astype(f)
    gmr = _rot(gum.reshape(T_, BK), T_, BK)
    length = inputs["length"][sl].astype(np.int64)
    Mt = (length[None, :] > (np.arange(T_) + 1)[:, None]).astype(f)  # [T, 8]
    mb = _rot(Mt, T_, BL)
    return {"memT": memT, "gp2": gp2, "gmr": gmr, "mb": mb}


def _make_in_maps(inputs, T_):
    shared = _prep_shared(inputs, T_)
    return [dict(shared, **_prep_core(inputs, ci, T_)) for ci in range(NCORES)]


LAST_RESULTS = None


def kernel(**inputs):
    global LAST_RESULTS
    from concourse.bass_utils import run_bass_kernel_spmd

    inputs = {k: np.asarray(v) for k, v in inputs.items()}
    in_maps = _make_in_maps(inputs, T)
    nc = _build_program(float(np.asarray(inputs["p2_b"]).reshape(-1)[0]), T)
    res = run_bass_kernel_spmd(nc, in_maps, core_ids=list(range(NCORES)))
    LAST_RESULTS = res
    idx = np.concatenate([r["out_idx"] for r in res.results], axis=0)
    lp = np.concatenate([r["out_lp"][:, 0] for r in res.results], axis=0)
    return idx.astype(np.int32), lp.astype(np.float32)


# revision 49
# speedup vs baseline: 1.2574x; 1.0169x over previous
"""Trainium2 Bass kernel for the Actor sampling module (nn_Actor_47588237640247).

Strategy: pure data-parallel across 8 NeuronCores (8 samples each). Per core:
  Phase 1: choice features cf = relu(l1(memory)) computed once into SBUF in a
           transposed layout cfT[h, (t, b, k)] (fp32, ~12.6 MB).
  Scan:    511 fully-unrolled steps, run as TWO interleaved independent chains
           (samples 0-3 / 4-7) so the engines overlap the chains' serial
           latencies. Everything lives transposed (feature dim on partitions,
           (b,k) on the free axis); per step and half:
             hid = relu(P1cf @ cf_t + P1h @ h + b)          (PE x2 + ACT relu)
             raw = p2 @ hid                                  (PE -> [1,24])
             noisy = max(raw, -p2b) + (gumbel+maskneg+p2b)   (DVE stt, staged out)
             onehot = (noisy >= rowmax)                      (DVE reduce + cmp)
             chosen = sum_k onehot * cf_t                    (PE bcast + DVE x2)
             gates  = bias + Wcomb @ h + Wcf @ chosen        (PE, gates transposed)
             LSTM cell                                       (ACT sigmoid/tanh + DVE)
           Per-step rows (noisy scores, onehot) are staged in [1, 8*48] blocks
           and DMA'd every 8 steps into rotated [128, 4*48] buffers.
  End:     logits = noisy - gumbel; batched log-softmax gather (exp/ln on ACT)
           + masked sum -> log_probs; argmax decoded from onehot -> idx.
Host side only reshapes/transposes inputs and bakes gumbel+mask+bias tensors.
"""

import os
import sys

import numpy as np

for _p in ("/opt/trn_rl_repo", "/root/.axon_site/_ro/trn_rl_repo"):
    if os.path.isdir(_p) and _p not in sys.path:
        sys.path.append(_p)

B, L, E, H, C = 64, 512, 256, 128, 128
T = L - 1  # 511
NCORES = 8
BL = B // NCORES  # 8 samples per core
K6 = 6
BK = BL * K6  # 48
HB = BL // 2  # 4 samples per half-chain
HK = HB * K6  # 24
NEG = np.float32(-1.0e30)


def _build_program(p2b_val: float, T_: int):
    import concourse.bass as bass
    import concourse.tile as tile
    from concourse import bacc, mybir

    f32 = mybir.dt.float32
    i32 = mybir.dt.int32
    AL = mybir.AluOpType
    AF = mybir.ActivationFunctionType
    AX = mybir.AxisListType

    NT_ = (T_ + 127) // 128  # rotated-layout column blocks

    nc = bacc.Bacc(
        "TRN2", target_bir_lowering=False, debug=False, num_devices=NCORES
    )

    d_memT = nc.dram_tensor("memT", [E, BL * T_], f32, kind="ExternalInput")
    d_gp2 = nc.dram_tensor("gp2", [1, T_ * BK], f32, kind="ExternalInput")
    d_gmr = nc.dram_tensor("gmr", [128, NT_ * BK], f32, kind="ExternalInput")
    d_mb = nc.dram_tensor("mb", [128, NT_ * BL], f32, kind="ExternalInput")
    d_kp = nc.dram_tensor("kp", [128, NT_ * BK], f32, kind="ExternalInput")
    d_l1wT = nc.dram_tensor("l1wT", [E, 6 * H], f32, kind="ExternalInput")
    d_l1b6 = nc.dram_tensor("l1b6", [128, 6], f32, kind="ExternalInput")
    d_p1cfT = nc.dram_tensor("p1cfT", [128, 128], f32, kind="ExternalInput")
    d_p1hT = nc.dram_tensor("p1hT", [128, 128], f32, kind="ExternalInput")
    d_p1b = nc.dram_tensor("p1b", [128, 1], f32, kind="ExternalInput")
    d_p2wT = nc.dram_tensor("p2wT", [128, 1], f32, kind="ExternalInput")
    d_wcfT = nc.dram_tensor("wcfT", [128, 4 * C], f32, kind="ExternalInput")
    d_wcoT = nc.dram_tensor("wcoT", [128, 4 * C], f32, kind="ExternalInput")
    d_bcr = nc.dram_tensor("bcr", [4, C], f32, kind="ExternalInput")
    d_bcT = nc.dram_tensor("bcT", [C, 4], f32, kind="ExternalInput")
    d_e46 = nc.dram_tensor("e46", [4, 4 * BK], f32, kind="ExternalInput")
    d_id128 = nc.dram_tensor("id128", [128, 128], f32, kind="ExternalInput")
    d_oidx = nc.dram_tensor("out_idx", [BL, T_], i32, kind="ExternalOutput")
    d_olp = nc.dram_tensor("out_lp", [BL, 1], f32, kind="ExternalOutput")

    with tile.TileContext(nc) as tc:
        with (
            tc.tile_pool(name="persist", bufs=1) as P,
            tc.tile_pool(name="blk", bufs=3) as BP,
            tc.tile_pool(name="work", bufs=4) as W,
        ):
            # ---- persistent SBUF tiles
            l1b6 = P.tile([128, 6], f32, tag="l1b6")
            cft = P.tile([128, T_, BK], f32, tag="cft")
            p1cf = P.tile([128, 128], f32, tag="p1cf")
            p1h = P.tile([128, 128], f32, tag="p1h")
            p1b = P.tile([128, 1], f32, tag="p1b")
            p2w = P.tile([128, 1], f32, tag="p2w")
            wcf = P.tile([128, 4 * C], f32, tag="wcf")
            wco = P.tile([128, 4 * C], f32, tag="wco")
            bcr4 = P.tile([4, C], f32, tag="bcr4")
            bcT = P.tile([C, 4], f32, tag="bcT")
            e46 = P.tile([4, 4 * BK], f32, tag="e46")
            id128 = P.tile([128, 128], f32, tag="id128")
            gmr = P.tile([128, NT_ * BK], f32, tag="gmr")
            mbs = P.tile([128, NT_ * BL], f32, tag="mbs")
            kps = P.tile([128, NT_ * BK], f32, tag="kps")
            scb = P.tile([128, NT_, BK], f32, tag="scb")
            oneb = P.tile([128, NT_, BK], f32, tag="oneb")

            ones1128 = P.tile([1, 128], f32, tag="ones1128")
            ones128 = P.tile([128, 1], f32, tag="ones128")
            zero128 = P.tile([128, 1], f32, tag="zero128")

            # ---- input DMAs
            nc.sync.dma_start(out=l1b6[:], in_=d_l1b6.ap())
            nc.sync.dma_start(out=p1cf[:], in_=d_p1cfT.ap())
            nc.sync.dma_start(out=p1h[:], in_=d_p1hT.ap())
            nc.sync.dma_start(out=p1b[:], in_=d_p1b.ap())
            nc.sync.dma_start(out=p2w[:], in_=d_p2wT.ap())
            nc.sync.dma_start(out=wcf[:], in_=d_wcfT.ap())
            nc.sync.dma_start(out=wco[:], in_=d_wcoT.ap())
            nc.sync.dma_start(out=bcr4[:], in_=d_bcr.ap())
            nc.sync.dma_start(out=bcT[:], in_=d_bcT.ap())
            nc.sync.dma_start(out=e46[:], in_=d_e46.ap())
            nc.sync.dma_start(out=id128[:], in_=d_id128.ap())
            nc.sync.dma_start(out=gmr[:], in_=d_gmr.ap())
            nc.sync.dma_start(out=mbs[:], in_=d_mb.ap())
            nc.sync.dma_start(out=kps[:], in_=d_kp.ap())


            nc.vector.memset(ones1128[:], 1.0)
            nc.vector.memset(ones128[:], 1.0)
            nc.vector.memset(zero128[:], 0.0)
            nc.vector.memset(scb[:], 0.0)
            nc.vector.memset(oneb[:], 0.0)

            # ---- phase 1: cf = relu(l1 @ mem + b), scattered into cfT layout
            with (
                tc.tile_pool(name="ph1sbuf", bufs=1) as P1S,
                tc.tile_pool(name="ph1psum", bufs=4, space="PSUM") as PP1,
            ):
                memT0 = P1S.tile([128, BL * T_], f32, tag="memT0")
                memT1 = P1S.tile([128, BL * T_], f32, tag="memT1")
                l1w0 = P1S.tile([128, 6 * H], f32, tag="l1w0")
                l1w1 = P1S.tile([128, 6 * H], f32, tag="l1w1")
                nc.sync.dma_start(out=memT0[:], in_=d_memT.ap()[0:128, :])
                nc.sync.dma_start(out=memT1[:], in_=d_memT.ap()[128:256, :])
                nc.sync.dma_start(out=l1w0[:], in_=d_l1wT.ap()[0:128, :])
                nc.sync.dma_start(out=l1w1[:], in_=d_l1wT.ap()[128:256, :])
                for b in range(BL):
                    for k in range(K6):
                        pC = PP1.tile([128, T_], f32, tag="pC")
                        nc.tensor.matmul(
                            pC[:],
                            l1w0[:, k * H : (k + 1) * H],
                            memT0[:, b * T_ : (b + 1) * T_],
                            start=True,
                            stop=False,
                        )
                        nc.tensor.matmul(
                            pC[:],
                            l1w1[:, k * H : (k + 1) * H],
                            memT1[:, b * T_ : (b + 1) * T_],
                            start=False,
                            stop=True,
                        )
                        nc.vector.tensor_scalar(
                            cft[:, :, b * K6 + k],
                            pC[:],
                            l1b6[:, k : k + 1],
                            0.0,
                            AL.add,
                            AL.max,
                        )

            # ---- scan: full-width chain; speculative LSTM over all 6 candidates
            # off-chain: gates_k = bias + Wco@h + (Wcf@cf_k pre-batched); LSTM for
            # all k; chain only: hid-relu -> scores -> noisy -> argmax -> select.
            MB = 16  # steps per staging block
            with tc.tile_pool(name="spsum", bufs=2, space="PSUM") as SP:
                hsel = W.tile([128, 2 * BL], f32, tag="hcsel", name="hsel0")
                nc.vector.memset(hsel[:], 0.0)
                csel0 = W.tile([128, BL], f32, tag="csel0", name="csel0")
                nc.vector.memset(csel0[:], 0.0)
                csel = csel0
                nb8 = None
                one16 = None
                gpblk = None
                gcf16 = None
                for t in range(T_):
                    m16 = t % MB
                    if m16 == 0:
                        nb = min(MB, T_ - t)
                        gpblk = BP.tile([1, MB * BK], f32, tag="gpblk")
                        nc.sync.dma_start(
                            out=gpblk[:, : nb * BK],
                            in_=d_gp2.ap()[:, t * BK : (t + nb) * BK],
                        )
                        nb8 = BP.tile([1, MB * BK], f32, tag="nb8")
                        one16 = BP.tile([1, MB * BK], f32, tag="one16")
                        # pre-batch Wcf @ cf for the block: gcf16[t', (g,b,k)]
                        gcf16 = BP.tile([128, MB, 4 * BK], f32, tag="gcf16", bufs=2)
                        for g in range(4):
                            for c2 in range(8):
                                t0c = c2 * 2
                                ch = min(2, nb - t0c)
                                if ch <= 0:
                                    continue
                                psC = SP.tile(
                                    [128, 8 * BK], f32, tag="psC", bufs=2, name="psC"
                                )
                                nc.tensor.matmul(
                                    psC[:, : ch * BK],
                                    wcf[:, g * C : (g + 1) * C],
                                    cft[:, t + t0c : t + t0c + ch, :].rearrange(
                                        "p a k -> p (a k)"
                                    ),
                                    start=True,
                                    stop=True,
                                )
                                nc.scalar.activation(
                                    gcf16[
                                        :, t0c : t0c + ch, g * BK : (g + 1) * BK
                                    ],
                                    psC[:, : ch * BK].rearrange(
                                        "p (a k) -> p a k", k=BK
                                    ),
                                    AF.Identity,
                                    bias=bcT[:, g : g + 1],
                                    scale=1.0,
                                )

                    cft_t = cft[:, t, :]  # [128, 48]
                    hT48b = (
                        hsel[:, 0:BL].unsqueeze(2).to_broadcast([128, BL, K6])
                    )
                    hT8 = hsel[:, 0:BL]

                    # hid pre-activation (chain) + gate psums (off-chain)
                    pA = SP.tile([128, BK], f32, tag="pA", bufs=2)
                    nc.tensor.matmul(pA[:], p1cf[:], cft_t, start=True, stop=False)
                    nc.tensor.matmul(
                        pA.rearrange("p (b k) -> p b k", k=K6),
                        p1h[:],
                        hT48b,
                        start=False,
                        stop=True,
                    )

                    pGifo = SP.tile([128, 3 * BK], f32, tag="pGifo", bufs=1)
                    pGg = SP.tile([128, BK], f32, tag="pGg", bufs=1)
                    nc.tensor.matmul(
                        pGifo[:],
                        id128[:],
                        gcf16[:, m16, 0 : 3 * BK],
                        start=True,
                        stop=False,
                    )
                    nc.tensor.matmul(
                        pGg[:],
                        id128[:],
                        gcf16[:, m16, 3 * BK : 4 * BK],
                        start=True,
                        stop=False,
                    )
                    hT8b = hT8.unsqueeze(2).to_broadcast([128, BL, K6])
                    for g in range(3):
                        nc.tensor.matmul(
                            pGifo[:, g * BK : (g + 1) * BK].rearrange(
                                "p (b k) -> p b k", k=K6
                            ),
                            wco[:, g * C : (g + 1) * C],
                            hT8b,
                            start=False,
                            stop=(g == 2),
                        )
                    nc.tensor.matmul(
                        pGg.rearrange("p (b k) -> p b k", k=K6),
                        wco[:, 3 * C : 4 * C],
                        hT8b,
                        start=False,
                        stop=True,
                    )

                    # speculative LSTM for all 6 candidates (off-chain)
                    # i/f/o weights pre-scaled 0.5 on host: sigmoid(x) =
                    # sigmoid-table with scale=2 on the halved gates.
                    sig = W.tile([128, 3 * BK], f32, tag="sig")
                    nc.scalar.activation(
                        sig[:], pGifo[:], AF.Sigmoid, bias=zero128[:], scale=2.0
                    )
                    th = W.tile([128, 4 * BK], f32, tag="th")
                    nc.scalar.activation(
                        th[:, 3 * BK : 4 * BK],
                        pGg[:],
                        AF.Tanh,
                        bias=zero128[:],
                        scale=1.0,
                    )
                    hc_cand = W.tile([128, 2 * BK], f32, tag="hc_cand")
                    t1 = W.tile([128, BK], f32, tag="t1")
                    nc.vector.tensor_mul(
                        t1.rearrange("p (b k) -> p b k", k=K6),
                        sig[:, BK : 2 * BK].rearrange("p (b k) -> p b k", k=K6),
                        csel.unsqueeze(2).to_broadcast([128, BL, K6]),
                    )
                    t2 = W.tile([128, BK], f32, tag="t2")
                    nc.vector.tensor_mul(t2[:], sig[:, 0:BK], th[:, 3 * BK : 4 * BK])
                    nc.vector.tensor_add(hc_cand[:, BK : 2 * BK], t1[:], t2[:])
                    tcs = W.tile([128, BK], f32, tag="tcs")
                    nc.scalar.activation(
                        tcs[:], hc_cand[:, BK : 2 * BK], AF.Tanh, bias=zero128[:], scale=1.0
                    )

                    # chain: hid -> scores -> noisy -> argmax -> select h,c
                    hid = W.tile([128, BK], f32, tag="hid")
                    nc.vector.tensor_scalar(
                        hid[:], pA[:], p1b[:], 0.0, AL.add, AL.max
                    )
                    pS = SP.tile([1, BK], f32, tag="pS", bufs=1)
                    nc.tensor.matmul(pS[:], p2w[:], hid[:], start=True, stop=True)
                    ms = slice(m16 * BK, (m16 + 1) * BK)
                    nc.vector.scalar_tensor_tensor(
                        nb8[:, ms], pS[:], -p2b_val, gpblk[:, ms], AL.max, AL.add
                    )
                    nv = nb8[:, ms].rearrange("p (b k) -> p b k", k=K6)
                    nmax = W.tile([1, BL], f32, tag="nmax")
                    nc.vector.reduce_max(nmax[:], nv, axis=AX.X)
                    one_v = one16[:, ms].rearrange("p (b k) -> p b k", k=K6)
                    nmax_b = nmax.unsqueeze(2).to_broadcast([1, BL, K6])
                    nc.vector.tensor_tensor(one_v, nv, nmax_b, op=AL.is_ge)
                    pO2 = SP.tile([128, 2 * BK], f32, tag="pO2", bufs=1)
                    nc.tensor.matmul(
                        pO2[:],
                        ones1128[:],
                        one16[:, ms]
                        .rearrange("p (a k) -> p a k", a=1)
                        .to_broadcast([1, 2, BK]),
                        start=True,
                        stop=True,
                    )
                    tmp96 = W.tile([128, 2 * BK], f32, tag="tmp96")
                    nc.vector.tensor_mul(
                        hc_cand[:, 0:BK], sig[:, 2 * BK : 3 * BK], pO2[:, 0:BK]
                    )
                    nc.vector.tensor_mul(
                        tmp96[:, BK : 2 * BK],
                        hc_cand[:, BK : 2 * BK],
                        pO2[:, BK : 2 * BK],
                    )
                    nc.vector.tensor_mul(tmp96[:, 0:BK], hc_cand[:, 0:BK], tcs[:])
                    hcsel = W.tile([128, 2 * BL], f32, tag="hcsel")
                    nc.vector.reduce_sum(
                        hcsel[:],
                        tmp96.rearrange("p (a b k) -> p a b k", a=2, k=K6),
                        axis=AX.X,
                    )
                    hsel = hcsel
                    csel = hcsel[:, BL : 2 * BL]

                    if m16 == MB - 1 or t == T_ - 1:
                        nb = m16 + 1
                        p0 = (t - m16) % 128
                        jj = (t - m16) // 128
                        nc.sync.dma_start(
                            out=scb[p0 : p0 + nb, jj, :],
                            in_=nb8[:, : nb * BK].rearrange(
                                "p (n k) -> p n k", k=BK
                            ),
                        )
                        nc.sync.dma_start(
                            out=oneb[p0 : p0 + nb, jj, :],
                            in_=one16[:, : nb * BK].rearrange(
                                "p (n k) -> p n k", k=BK
                            ),
                        )

            # ---- end phase: log-probs and indices
            with (
                tc.tile_pool(name="endp", bufs=1) as EP,
                tc.tile_pool(name="endpsum", bufs=1, space="PSUM") as PEP,
            ):
                scb_f = scb.rearrange("p j k -> p (j k)")
                oneb_f = oneb.rearrange("p j k -> p (j k)")
                logits = EP.tile([128, NT_ * BK], f32, tag="logits")
                nc.vector.tensor_sub(logits[:], scb_f, gmr[:])
                expv = EP.tile([128, NT_ * BK], f32, tag="expv")
                nc.scalar.activation(
                    expv[:], logits[:], AF.Exp, bias=zero128[:], scale=1.0
                )
                esum = EP.tile([128, NT_ * BL], f32, tag="esum")
                nc.vector.reduce_sum(
                    esum[:],
                    expv.rearrange("p (j b k) -> p j b k", b=BL, k=K6),
                    axis=AX.X,
                )
                lse = EP.tile([128, NT_ * BL], f32, tag="lse")
                nc.scalar.activation(
                    lse[:], esum[:], AF.Ln, bias=zero128[:], scale=1.0
                )
                selp = EP.tile([128, NT_ * BK], f32, tag="selp")
                nc.vector.tensor_mul(selp[:], logits[:], oneb_f)
                sel = EP.tile([128, NT_ * BL], f32, tag="sel")
                nc.vector.reduce_sum(
                    sel[:],
                    selp.rearrange("p (j b k) -> p j b k", b=BL, k=K6),
                    axis=AX.X,
                )
                diff = EP.tile([128, NT_ * BL], f32, tag="diff")
                nc.vector.tensor_sub(diff[:], sel[:], lse[:])
                masked = EP.tile([128, NT_ * BL], f32, tag="masked")
                nc.vector.tensor_mul(masked[:], diff[:], mbs[:])
                part = EP.tile([128, BL], f32, tag="part")
                nc.vector.reduce_sum(
                    part[:],
                    masked.rearrange("p (j b) -> p b j", b=BL),
                    axis=AX.X,
                )
                accp = PEP.tile([BL, 1], f32, tag="accp")
                nc.tensor.matmul(accp[:], part[:], ones128[:], start=True, stop=True)
                lp_sb = EP.tile([BL, 1], f32, tag="lp_sb")
                nc.vector.tensor_copy(lp_sb[:], accp[:])
                nc.sync.dma_start(out=d_olp.ap(), in_=lp_sb[:])

                ksel = EP.tile([128, NT_ * BK], f32, tag="ksel")
                nc.vector.tensor_mul(ksel[:], oneb_f, kps[:])
                idxf = EP.tile([128, NT_ * BL], f32, tag="idxf")
                nc.vector.reduce_sum(
                    idxf[:],
                    ksel.rearrange("p (j b k) -> p j b k", b=BL, k=K6),
                    axis=AX.X,
                )
                idxi = EP.tile([128, NT_ * BL], i32, tag="idxi")
                nc.vector.tensor_copy(idxi[:], idxf[:])
                for j in range(NT_):
                    pmax = min(128, T_ - j * 128)
                    nc.sync.dma_start(
                        out=d_oidx.ap()[:, j * 128 : j * 128 + pmax].transpose(
                            [1, 0]
                        ),
                        in_=idxi[0:pmax, j * BL : (j + 1) * BL],
                    )

    nc.compile()
    return nc


def _prep_shared(inputs, T_):
    """Host-side weight prep (shared across cores)."""
    f = np.float32
    l1_w = inputs["l1_w"].astype(f)
    p1_w = inputs["p1_w"].astype(f)
    w_ih = inputs["w_ih"].astype(f)
    w_hh = inputs["w_hh"].astype(f)
    bc = (inputs["b_ih"] + inputs["b_hh"]).astype(f)
    NT_ = (T_ + 127) // 128
    gorder = [0, 1, 3, 2]  # i, f, o, g
    wcfT = np.empty((128, 4 * C), f)
    wcoT = np.empty((128, 4 * C), f)
    bcr = np.empty((4, C), f)
    for gi, g in enumerate(gorder):
        rows = slice(g * C, (g + 1) * C)
        s5 = f(0.5) if gi < 3 else f(1.0)  # sigmoid-as-tanh pre-scale (i,f,o)
        wcfT[:, gi * C : (gi + 1) * C] = s5 * w_ih[rows, :H].T
        wcoT[:, gi * C : (gi + 1) * C] = s5 * (w_ih[rows, H:] + w_hh[rows, :]).T
        bcr[gi, :] = s5 * bc[rows]
    kcol = np.tile(np.arange(K6, dtype=f), BL)  # [48]
    kp = np.broadcast_to(kcol, (128, NT_, BK)).reshape(128, NT_ * BK).copy()
    bcT = bcr.T.copy()
    return {
        "bcT": bcT,
        "l1wT": np.ascontiguousarray(l1_w.T),
        "l1b6": np.ascontiguousarray(inputs["l1_b"].astype(f).reshape(6, 128).T),
        "p1cfT": np.ascontiguousarray(p1_w[:, :H].T),
        "p1hT": np.ascontiguousarray(p1_w[:, H:].T),
        "p1b": inputs["p1_b"].astype(f).reshape(128, 1).copy(),
        "p2wT": np.ascontiguousarray(inputs["p2_w"].astype(f).T),
        "wcfT": wcfT,
        "wcoT": wcoT,
        "bcr": bcr,
        "e46": np.kron(np.eye(4, dtype=f), np.ones((1, BK), f)),
        "id128": np.eye(128, dtype=f),
        "kp": kp,
    }


def _rot(x_t, T_, width):
    """[T, width] -> rotated [128, NT*width] (partition = t%128, block = t//128)."""
    f = np.float32
    NT_ = (T_ + 127) // 128
    Xp = np.zeros((NT_ * 128, width), f)
    Xp[:T_] = x_t
    return np.ascontiguousarray(
        Xp.reshape(NT_, 128, width).transpose(1, 0, 2).reshape(128, NT_ * width)
    )


def _prep_core(inputs, ci, T_):
    f = np.float32
    p2b = f(np.asarray(inputs["p2_b"]).reshape(-1)[0])
    sl = slice(ci * BL, (ci + 1) * BL)
    mem = inputs["memory"][sl, 2 : 2 + T_, :].astype(f)  # [8, T, 256]
    memT = np.ascontiguousarray(mem.transpose(2, 0, 1).reshape(E, BL * T_))
    mask_t = inputs["mask"][sl, 1 : 1 + T_, :]  # [8, T, 6]
    SMt = np.where(mask_t, f(0.0), NEG).astype(f).transpose(1, 0, 2)  # [T, 8, 6]
    gum = inputs["gumbel"][:T_, sl, :].astype(f)  # [T, 8, 6]
    gp2 = (gum + SMt + p2b).reshape(1, T_ * BK).astype(f)
    gmr = _rot(gum.reshape(T_, BK), T_, BK)
    length = inputs["length"][sl].astype(np.int64)
    Mt = (length[None, :] > (np.arange(T_) + 1)[:, None]).astype(f)  # [T, 8]
    mb = _rot(Mt, T_, BL)
    return {"memT": memT, "gp2": gp2, "gmr": gmr, "mb": mb}


def _make_in_maps(inputs, T_):
    shared = _prep_shared(inputs, T_)
    return [dict(shared, **_prep_core(inputs, ci, T_)) for ci in range(NCORES)]


LAST_RESULTS = None


def kernel(**inputs):
    global LAST_RESULTS
    from concourse.bass_utils import run_bass_kernel_spmd

    inputs = {k: np.asarray(v) for k, v in inputs.items()}
    in_maps = _make_in_maps(inputs, T)
    nc = _build_program(float(np.asarray(inputs["p2_b"]).reshape(-1)[0]), T)
    res = run_bass_kernel_spmd(nc, in_maps, core_ids=list(range(NCORES)))
    LAST_RESULTS = res
    idx = np.concatenate([r["out_idx"] for r in res.results], axis=0)
    lp = np.concatenate([r["out_lp"][:, 0] for r in res.results], axis=0)
    return idx.astype(np.int32), lp.astype(np.float32)
